# revision 18
# baseline (speedup 1.0000x reference)
"""Trainium2 8-core kernel for a GPT-style transformer block (v3).

Strategy:
  - Megatron-style QKV: every core holds the FULL (quantized) x and the QKV
    weight columns of its own 2 heads, so Q^T/K^T/V for all 4096 tokens are
    computed locally -- no pre-attention collectives at all.
  - LN1 is folded algebraically into the QKV matmul: qkv = rstd*(x@W) -
    mu*rstd*colsum(W). Per-token stats (sum x, sum x^2) are computed on each
    core's own 512-token span and shared with one tiny (4KB) AllGather.
  - QKV and proj matmuls run in fp8 (e4m3) DoubleRow mode: K=256 per
    instruction, 2x PE throughput. fc/mlp stay bf16 (fp8 fails tolerance).
  - Attention (2 heads/core, all tokens) processes query-chunk pairs with
    shared key tiles; softmax denominators ride along as a ones-column in V;
    normalization uses vector reciprocal (no scalar table switches).
  - y returns token-sharded via one fp8 AllToAll (0.5MB); proj + residual +
    LN2 + MLP are local to the core's 512 tokens.
"""

import sys

sys.path.insert(0, "/opt/trn_rl_repo")

import numpy as np
import ml_dtypes

import concourse.bass as bass
import concourse.mybir as mybir
import concourse.tile as tile
from concourse import bacc, bass_utils

BF16 = mybir.dt.bfloat16
F32 = mybir.dt.float32
FP8 = mybir.dt.float8e4
AF = mybir.ActivationFunctionType
ALU = mybir.AluOpType
DR = mybir.MatmulPerfMode.DoubleRow
NP_BF16 = ml_dtypes.bfloat16
NP_FP8 = ml_dtypes.float8_e4m3

B, T, C, H, HS, FF = 2, 2048, 1024, 16, 64, 4096
CORES = 8
S = 512            # tokens per core (MLP/residual shard)
TT = B * T         # 4096 total tokens
NCT = C // 128     # 8 feature tiles
NDR = NCT // 2     # 4 DoubleRow contraction pairs
NFT = FF // 128    # 32 mlp hidden tiles
QCH = 256          # query chunk
NQC = T // QCH     # 8 query chunks per batch
NKT = T // 128     # 16 key tiles per batch
NTT = TT // 128    # 32 token tiles total
EPS = 1e-5
SY = 30.0          # fixed y quantization scale (|y| < 8 by softmax convexity)


def build(flags):
    (use_bqkv, use_bproj, use_bfc, use_bmlp, debug) = flags

    nc = bacc.Bacc("TRN2", target_bir_lowering=False, debug=False,
                   num_devices=CORES)

    # ---------------- DRAM inputs ----------------
    xq = nc.dram_tensor("xq", [128, NCT, TT], FP8, kind="ExternalInput")
    xt = nc.dram_tensor("xt", [C, S], BF16, kind="ExternalInput")
    xt32 = nc.dram_tensor("xt32", [C, S], F32, kind="ExternalInput")
    wq8 = nc.dram_tensor("wq8", [128, NCT, 384], FP8, kind="ExternalInput")
    wsum3 = nc.dram_tensor("wsum3", [128, 3], F32, kind="ExternalInput")
    wvsum = nc.dram_tensor("wvsum", [1, 128], F32, kind="ExternalInput")
    wp8 = nc.dram_tensor("wp8", [128, NCT, C], FP8, kind="ExternalInput")
    scales = nc.dram_tensor("scales", [1, 8], F32, kind="ExternalInput")
    b_qkv = nc.dram_tensor("b_qkv", [128, 3], F32, kind="ExternalInput")
    bv_row = nc.dram_tensor("bv_row", [1, 128], F32, kind="ExternalInput")
    w_fc = nc.dram_tensor("w_fc", [C, FF], BF16, kind="ExternalInput")
    w_mlp = nc.dram_tensor("w_mlp", [FF, C], BF16, kind="ExternalInput")
    b_proj = nc.dram_tensor("b_proj", [128, NCT], F32, kind="ExternalInput")
    b_fc = nc.dram_tensor("b_fc", [128, NFT], F32, kind="ExternalInput")
    b_mlp = nc.dram_tensor("b_mlp", [128, NCT], F32, kind="ExternalInput")
    mask0_d = nc.dram_tensor("mask0", [128, 4 * QCH], BF16, kind="ExternalInput")
    mask1_d = nc.dram_tensor("mask1", [128, 4 * QCH], BF16, kind="ExternalInput")
    mask0x_d = nc.dram_tensor("mask0x", [128, 2 * QCH], BF16, kind="ExternalInput")
    mask1x_d = nc.dram_tensor("mask1x", [128, 2 * QCH], BF16, kind="ExternalInput")
    out_d = nc.dram_tensor("out", [C, S], F32, kind="ExternalOutput")
    dbg = {}
    if debug:
        for nm, shp, dt in [("d_qt", [128, TT], BF16), ("d_kt", [128, TT], BF16),
                            ("d_v", [TT, 128], BF16), ("d_yt", [128, TT], FP8),
                            ("d_x2", [C, S], F32), ("d_ln2", [C, S], BF16)]:
            dbg[nm] = nc.dram_tensor(nm, shp, dt, kind="ExternalOutput")

    with tile.TileContext(nc) as tc:
        _build_body(nc, tc, locals(), flags)
    nc.compile()
    return nc


def _build_body(nc, tc, t_, flags):
    (use_bqkv, use_bproj, use_bfc, use_bmlp, debug) = flags
    xq, xt, xt32 = t_["xq"], t_["xt"], t_["xt32"]
    wq8, wsum3, wvsum, wp8 = t_["wq8"], t_["wsum3"], t_["wvsum"], t_["wp8"]
    scales, b_qkv, bv_row = t_["scales"], t_["b_qkv"], t_["bv_row"]
    w_fc, w_mlp = t_["w_fc"], t_["w_mlp"]
    b_proj, b_fc, b_mlp = t_["b_proj"], t_["b_fc"], t_["b_mlp"]
    mask0_d, mask1_d = t_["mask0_d"], t_["mask1_d"]
    mask0x_d, mask1x_d = t_["mask0x_d"], t_["mask1x_d"]
    out_d, dbg = t_["out_d"], t_["dbg"]

    from contextlib import ExitStack
    from concourse.tile import add_dep_helper

    def _delay_after(frm, to):
        f = frm.ins if hasattr(frm, "ins") else frm
        t = to.ins if hasattr(to, "ins") else to
        add_dep_helper(t, f, sync=True, reason="delay heavy DMA")
    es = ExitStack()

    consts = es.enter_context(tc.tile_pool(name="consts", bufs=1))
    dram = es.enter_context(tc.tile_pool(name="dram", bufs=1, space="DRAM"))
    # q/k/v activation tiles live from QKV production through attention
    qkvt_cm = tc.tile_pool(name="qkvt_p", bufs=1, side="right")
    qkvt_p = qkvt_cm.__enter__()  # closed manually after attention
    x2t_p = es.enter_context(tc.tile_pool(name="x2t_p", bufs=1))

    # ---- constants ----
    ones_col = consts.tile([128, 1], BF16, name="ones_col")
    nc.vector.memset(ones_col, 1.0)
    ones_row = consts.tile([1, 128], BF16, name="ones_row")
    nc.vector.memset(ones_row, 1.0)
    eps_t = consts.tile([1, 1], F32, name="eps_t")
    nc.vector.memset(eps_t, EPS)
    mask0 = consts.tile([128, 4 * QCH], BF16, name="mask0")
    nc.sync.dma_start(out=mask0, in_=mask0_d[:, :])
    mask1 = consts.tile([128, 4 * QCH], BF16, name="mask1")
    nc.sync.dma_start(out=mask1, in_=mask1_d[:, :])
    mask0x = consts.tile([128, 2 * QCH], BF16, name="mask0x")
    nc.sync.dma_start(out=mask0x, in_=mask0x_d[:, :])
    mask1x = consts.tile([128, 2 * QCH], BF16, name="mask1x")
    nc.sync.dma_start(out=mask1x, in_=mask1x_d[:, :])
    sc_sb = consts.tile([1, 8], F32, name="sc_sb")
    nc.sync.dma_start(out=sc_sb, in_=scales[:, :])
    wsum_sb = consts.tile([128, 3], F32, name="wsum_sb")
    nc.sync.dma_start(out=wsum_sb, in_=wsum3[:, :])
    wvsum_sb = consts.tile([1, 128], F32, name="wvsum_sb")
    nc.sync.dma_start(out=wvsum_sb, in_=wvsum[:, :])
    wvsum_b = consts.tile([128, 128], F32, name="wvsum_b")
    nc.gpsimd.partition_broadcast(wvsum_b, wvsum_sb)
    kproj_b = consts.tile([128, 1], F32, name="kproj_b")
    nc.gpsimd.partition_broadcast(kproj_b, sc_sb[0:1, 1:2])
    if use_bqkv:
        bqkv_sb = consts.tile([128, 3], F32, name="bqkv_sb")
        nc.sync.dma_start(out=bqkv_sb, in_=b_qkv[:, :])
        bv_sb = consts.tile([1, 128], F32, name="bv_sb")
        nc.sync.dma_start(out=bv_sb, in_=bv_row[:, :])
        bv_b = consts.tile([128, 128], F32, name="bv_b")
        nc.gpsimd.partition_broadcast(bv_b, bv_sb)
    bproj_sb = (consts.tile([128, NCT], F32, name="bproj_sb")
                if use_bproj else None)
    if use_bproj:
        nc.sync.dma_start(out=bproj_sb, in_=b_proj[:, :])
    bfc_sb = consts.tile([128, NFT], F32, name="bfc_sb") if use_bfc else None
    if use_bfc:
        nc.sync.dma_start(out=bfc_sb, in_=b_fc[:, :])
    bmlp_sb = consts.tile([128, NCT], F32, name="bmlp_sb") if use_bmlp else None
    if use_bmlp:
        nc.sync.dma_start(out=bmlp_sb, in_=b_mlp[:, :])

    # ---- collective DRAM tiles ----
    cc0_in = dram.tile([1, 128], BF16, name="cc0_in")
    cc0_out = dram.tile([CORES, 128], BF16, name="cc0_out")
    st_in = dram.tile([2, 512], F32, name="st_in")
    st_out = dram.tile([CORES, 2, 512], F32, name="st_out")
    rr0 = dram.tile([1, TT], F32, name="rr0")
    rr1 = dram.tile([1, TT], F32, name="rr1")
    rr0b = dram.tile([1, TT], BF16, name="rr0b")
    rr1b = dram.tile([1, TT], BF16, name="rr1b")
    cc3_in = dram.tile([CORES, 128, S], FP8, name="cc3_in")
    cc3_out = dram.tile([CORES, 128, S], FP8, name="cc3_out")

    # comms warmup
    nc.sync.dma_start(out=cc0_in, in_=mask0_d[0:1, 0:128])
    nc.gpsimd.collective_compute(
        "AllGather", ALU.bypass,
        replica_groups=[list(range(CORES))],
        ins=[cc0_in[:, :].opt()],
        outs=[cc0_out[:, :].opt()])

    # persistent attention inputs
    qtb = [qkvt_p.tile([128, T], BF16, name=f"qtb_{b}") for b in range(B)]
    ktb = [qkvt_p.tile([128, T], BF16, name=f"ktb_{b}") for b in range(B)]
    vt = [qkvt_p.tile([128, 2, 128], BF16, name=f"vt_{tt}")
          for tt in range(NTT)]
    for v3 in vt:
        nc.gpsimd.memset(v3[:, :, 0:1], 1.0)
        nc.gpsimd.memset(v3[:, :, 1:64], 0.0)

    # =========================================================
    # Phase B: LN1 statistics on my span + AllGather
    # =========================================================
    qkv_cm = tc.tile_pool(name="qkv_pool", bufs=1)
    qp = qkv_cm.__enter__()
    psQ_cm = tc.tile_pool(name="psQ", bufs=1, space="PSUM")
    psQ = psQ_cm.__enter__()

    s_ps = psQ.tile([1, 512], F32, name="s_ps", tag="st", bufs=2)
    q_ps = psQ.tile([1, 512], F32, name="q_ps", tag="st", bufs=2)
    xt_sb = []
    for c in range(NCT):
        tl = qp.tile([128, S], BF16, name=f"xt_{c}")
        nc.sync.dma_start(out=tl, in_=xt[c * 128:(c + 1) * 128, :])
        xt_sb.append(tl)
    for c in range(NCT):
        sq = qp.tile([128, S], BF16, name=f"sq_{c}", tag="sq", bufs=3)
        nc.vector.tensor_mul(sq, xt_sb[c], xt_sb[c])
        nc.tensor.matmul(s_ps[:, :], ones_col[:, :], xt_sb[c][:, :],
                         start=(c == 0), stop=(c == NCT - 1))
        nc.tensor.matmul(q_ps[:, :], ones_col[:, :], sq[:, :],
                         start=(c == 0), stop=(c == NCT - 1))
    s_sb = qp.tile([1, 512], F32, name="s_sb")
    nc.scalar.copy(s_sb, s_ps[:, :])
    q_sb = qp.tile([1, 512], F32, name="q_sb")
    nc.scalar.copy(q_sb, q_ps[:, :])
    nc.sync.dma_start(out=st_in[0:1, :], in_=s_sb)
    nc.sync.dma_start(out=st_in[1:2, :], in_=q_sb)
    ag_st = nc.gpsimd.collective_compute(
        "AllGather", ALU.bypass,
        replica_groups=[list(range(CORES))],
        ins=[st_in[:, :].opt()],
        outs=[st_out[:, :, :].opt()])

    # stats math on [8, 512] tiles: row j = token span j
    ag_sb = qp.tile([CORES, 2, 512], F32, name="ag_sb")
    nc.sync.dma_start(out=ag_sb, in_=st_out[:, :, :])
    sx8 = ag_sb[:, 0, :]
    sq8 = ag_sb[:, 1, :]
    kqk8 = qp.tile([CORES, 1], F32, name="kqk8")
    nc.gpsimd.partition_broadcast(kqk8, sc_sb[0:1, 0:1])
    eps8 = qp.tile([CORES, 1], F32, name="eps8")
    nc.gpsimd.partition_broadcast(eps8, eps_t)
    mu8 = qp.tile([CORES, 512], F32, name="mu8")
    nc.scalar.mul(mu8, sx8, 1.0 / C)
    msq8 = qp.tile([CORES, 512], F32, name="msq8")
    nc.scalar.mul(msq8, sq8, 1.0 / C)
    var8 = qp.tile([CORES, 512], F32, name="var8")
    nc.vector.tensor_mul(var8, mu8, mu8)
    nc.vector.tensor_sub(var8, msq8, var8)
    lnv8 = qp.tile([CORES, 512], F32, name="lnv8")
    nc.scalar.activation(lnv8, var8, AF.Ln, bias=eps8, scale=1.0)
    rstd8 = qp.tile([CORES, 512], F32, name="rstd8")
    nc.scalar.activation(rstd8, lnv8, AF.Exp, scale=-0.5)
    # rqk = rstd * (1/(Sx*Sw)) : dequant folded into the normalize scale
    rqk8 = qp.tile([CORES, 512], F32, name="rqk8")
    nc.vector.tensor_scalar(out=rqk8, in0=rstd8, scalar1=kqk8[:, 0:1],
                            scalar2=None, op0=ALU.mult)
    rqk8_bf = qp.tile([CORES, 512], BF16, name="rqk8_bf")
    nc.vector.tensor_copy(rqk8_bf, rqk8)
    nmurs8 = qp.tile([CORES, 512], F32, name="nmurs8")
    nc.vector.tensor_mul(nmurs8, mu8, rstd8)
    nc.scalar.mul(nmurs8, nmurs8, -1.0)
    nmurs8_bf = qp.tile([CORES, 512], BF16, name="nmurs8_bf")
    nc.vector.tensor_copy(nmurs8_bf, nmurs8)
    # column (token-on-partition) forms for the V eviction, via DRAM
    for j in range(CORES):
        nc.sync.dma_start(out=rr0[0:1, j * 512:(j + 1) * 512],
                          in_=rqk8[j:j + 1, :])
        nc.sync.dma_start(out=rr1[0:1, j * 512:(j + 1) * 512],
                          in_=nmurs8[j:j + 1, :])
        nc.sync.dma_start(out=rr0b[0:1, j * 512:(j + 1) * 512],
                          in_=rqk8_bf[j:j + 1, :])
        nc.sync.dma_start(out=rr1b[0:1, j * 512:(j + 1) * 512],
                          in_=nmurs8_bf[j:j + 1, :])
    rcol = qp.tile([128, NTT], F32, name="rcol")
    nc.sync.dma_start(out=rcol,
                      in_=rr0.rearrange("o (j p) -> (o p) j", p=128))
    ncol = qp.tile([128, NTT], F32, name="ncol")
    nc.sync.dma_start(out=ncol,
                      in_=rr1.rearrange("o (j p) -> (o p) j", p=128))
    # partition-0 rows for the per-chunk broadcasts
    rqk_row = qp.tile([1, TT], BF16, name="rqk_row")
    nc.sync.dma_start(out=rqk_row, in_=rr0b[0:1, :])
    nm_row = qp.tile([1, TT], BF16, name="nm_row")
    nc.sync.dma_start(out=nm_row, in_=rr1b[0:1, :])

    # =========================================================
    # Phase C: QKV in fp8 DoubleRow
    # =========================================================
    wq_sb = qp.tile([128, NCT, 384], FP8, name="wq_sb")
    nc.sync.dma_start(out=wq_sb, in_=wq8[:, :, :])

    for ch in range(NCT):
        cols = slice(ch * 512, (ch + 1) * 512)
        bi, r4 = ch // 4, ch % 4
        xc = qp.tile([128, NCT, 512], FP8, name=f"xc_{ch}", tag="xc", bufs=3)
        nc.sync.dma_start(out=xc, in_=xq[:, :, cols])
        rqk_b = qp.tile([128, 512], BF16, name=f"rqkb_{ch}", tag="rqkb",
                        bufs=2)
        nc.gpsimd.partition_broadcast(rqk_b, rqk_row[0:1, cols])
        nm_b = qp.tile([128, 512], BF16, name=f"nmb_{ch}", tag="nmb", bufs=2)
        nc.gpsimd.partition_broadcast(nm_b, nm_row[0:1, cols])
        for o in range(2):  # 0=q, 1=k
            ps = psQ.tile([128, 512], F32, name=f"qk_{ch}_{o}", tag="qk",
                          bufs=3)
            for d in range(NDR):
                nc.tensor.matmul(ps,
                                 wq_sb[:, 2 * d:2 * d + 2,
                                       o * 128:(o + 1) * 128],
                                 xc[:, 2 * d:2 * d + 2, :],
                                 start=(d == 0), stop=(d == NDR - 1),
                                 perf_mode=DR)
            tmp = qp.tile([128, 512], BF16, name=f"qkt_{ch}_{o}", tag="qkt",
                          bufs=3)
            nc.vector.tensor_mul(tmp, ps, rqk_b)
            dst = (qtb if o == 0 else ktb)[bi][:, r4 * 512:(r4 + 1) * 512]
            nc.vector.scalar_tensor_tensor(
                out=dst, in0=nm_b, scalar=wsum_sb[:, o:o + 1],
                in1=tmp, op0=ALU.mult, op1=ALU.add)
            if use_bqkv:
                nc.vector.tensor_scalar(
                    out=dst, in0=dst, scalar1=bqkv_sb[:, o:o + 1],
                    scalar2=None, op0=ALU.add)
        for t4 in range(4):
            ttile = ch * 4 + t4
            tc_sl = slice(t4 * 128, (t4 + 1) * 128)
            psv = psQ.tile([128, 128], F32, name=f"vps_{ttile}", tag="vps",
                           bufs=3)
            for d in range(NDR):
                nc.tensor.matmul(psv,
                                 xc[:, 2 * d:2 * d + 2, tc_sl],
                                 wq_sb[:, 2 * d:2 * d + 2, 256:384],
                                 start=(d == 0), stop=(d == NDR - 1),
                                 perf_mode=DR)
            outer = qp.tile([128, 128], F32, name=f"outer_{ttile}",
                            tag="outer", bufs=3)
            nc.vector.tensor_scalar(
                out=outer, in0=wvsum_b, scalar1=ncol[:, ttile:ttile + 1],
                scalar2=None, op0=ALU.mult)
            if use_bqkv:
                nc.vector.tensor_add(outer, outer, bv_b)
            v3 = vt[ttile]
            for h in range(2):
                nc.vector.scalar_tensor_tensor(
                    out=v3[:, h, 64:128], in0=psv[:, h * 64:(h + 1) * 64],
                    scalar=rcol[:, ttile:ttile + 1], in1=outer[:, h * 64:(h + 1) * 64],
                    op0=ALU.mult, op1=ALU.add)
        if debug:
            for o, nm in ((0, "d_qt"), (1, "d_kt")):
                nc.sync.dma_start(
                    out=dbg[nm][:, ch * 512:(ch + 1) * 512],
                    in_=(qtb if o == 0 else ktb)[bi][:, r4 * 512:(r4 + 1) * 512])
    if debug:
        for ttile in range(NTT):
            for h in range(2):
                nc.sync.dma_start(
                    out=dbg["d_v"][ttile * 128:(ttile + 1) * 128,
                                   h * 64:(h + 1) * 64],
                    in_=vt[ttile][:, h, 64:128])
    psQ_cm.__exit__(None, None, None)
    qkv_cm.__exit__(None, None, None)

    # =========================================================
    # Phase D: attention (my 2 heads, all tokens)
    # =========================================================
    att_pool = tc.tile_pool(name="att_pool", bufs=1)
    psB_pool = tc.tile_pool(name="psB", bufs=2, space="PSUM")
    W2 = 2 * QCH
    gate_anchor = [None]
    a2a_y = None
    with att_pool as ap, psB_pool as psB:

        def flush_normalize(item):
            fb, fp, y_A, y_B = item
            j = 4 * fb + fp
            for hi, y_ps in ((0, y_A), (1, y_B)):
                rec = ap.tile([1, W2], F32, name=f"rec_{fb}_{fp}_{hi}",
                              tag="rec", bufs=4)
                nc.vector.reciprocal(rec, y_ps[0:1, :])
                rec_bf = ap.tile([1, W2], BF16, name=f"recbf_{fb}_{fp}_{hi}",
                                 tag="recbf", bufs=4)
                nc.scalar.mul(rec_bf, rec, SY)
                den = ap.tile([128, W2], BF16, name=f"den_{fb}_{fp}_{hi}",
                              tag="den", bufs=4)
                nc.gpsimd.partition_broadcast(den, rec_bf)
                yt8 = ap.tile([128, W2], FP8, name=f"yt8_{fb}_{fp}_{hi}",
                              tag="yt8", bufs=4)
                nc.vector.tensor_mul(yt8[64:128, :], y_ps[64:128, :],
                                     den[64:128, :])
                nc.sync.dma_start(out=cc3_in[j, hi * 64:(hi + 1) * 64, :],
                                  in_=yt8[64:128, :])

        for b in range(B):
            for p in reversed(range(NQC // 2)):
                qc = 2 * p
                qs = qc * QCH
                nsh = 2 * (qc + 1)
                y_A = psB.tile([128, W2], F32, name=f"yA_{b}_{p}", tag="ya",
                               bufs=2)
                y_B = psB.tile([128, W2], F32, name=f"yB_{b}_{p}", tag="yb",
                               bufs=2)
                for kt in range(nsh + 2):
                    shared = kt < nsh
                    cols = slice(0, W2) if shared else slice(QCH, W2)
                    ncols = W2 if shared else QCH
                    s_AB = psB.tile([128, 2 * W2], F32, name=f"s_{b}_{p}_{kt}",
                                    tag="ps2", bufs=2)
                    nc.tensor.matmul(s_AB[:, 0:ncols],
                                     ktb[b][0:64, kt * 128:(kt + 1) * 128],
                                     qtb[b][0:64, qs + cols.start:qs + W2],
                                     start=True, stop=True)
                    nc.tensor.matmul(s_AB[:, W2:W2 + ncols],
                                     ktb[b][64:128, kt * 128:(kt + 1) * 128],
                                     qtb[b][64:128, qs + cols.start:qs + W2],
                                     start=True, stop=True)
                    e_AB = ap.tile([128, 2 * W2], BF16, name=f"e_{b}_{p}_{kt}",
                                   tag="eAB", bufs=5)
                    if shared:
                        nc.scalar.activation(e_AB, s_AB[:, :], AF.Exp,
                                             scale=1.0 / np.sqrt(HS))
                        if kt in (qc * 2, qc * 2 + 1):
                            mx = mask0x if kt == qc * 2 else mask1x
                            e4 = e_AB.rearrange("p (h c q) -> p h c q",
                                                h=2, c=2)
                            m4 = mx.rearrange("p (h q) -> p h q", h=2)
                            nc.vector.tensor_mul(e4[:, :, 0, :],
                                                 e4[:, :, 0, :], m4)
                    else:
                        e3 = e_AB.rearrange("p (h q) -> p h q", h=2)
                        s3 = s_AB.rearrange("p (h q) -> p h q", h=2)
                        nc.scalar.activation(e3[:, :, 0:QCH], s3[:, :, 0:QCH],
                                             AF.Exp, scale=1.0 / np.sqrt(HS))
                        mx = mask0x if kt == nsh else mask1x
                        nc.vector.tensor_mul(
                            e3[:, :, 0:QCH], e3[:, :, 0:QCH],
                            mx.rearrange("p (h q) -> p h q", h=2))
                    v3 = vt[b * NKT + kt]
                    nc.tensor.matmul(y_A[:, cols], v3[:, 0, :],
                                     e_AB[:, 0:ncols],
                                     start=(kt == 0), stop=(kt == nsh + 1),
                                     skip_group_check=True)
                    mmB = nc.tensor.matmul(y_B[:, cols], v3[:, 1, :],
                                           e_AB[:, W2:W2 + ncols],
                                           start=(kt == 0),
                                           stop=(kt == nsh + 1),
                                           skip_group_check=True)
                    if p == 3 and kt == nsh + 1:
                        gate_anchor.append(mmB)
                flush_normalize((b, p, y_A, y_B))
        gate_anchor[0] = gate_anchor[1]
        a2a_y = nc.gpsimd.collective_compute(
            "AllToAll", ALU.bypass,
            replica_groups=[list(range(CORES))],
            ins=[cc3_in[:, :, :].opt()],
            outs=[cc3_out[:, :, :].opt()])
    if debug:
        for j in range(CORES):
            nc.sync.dma_start(out=dbg["d_yt"][:, j * 512:(j + 1) * 512],
                              in_=cc3_in[j, :, :])
    qkvt_cm.__exit__(None, None, None)  # free qtb/ktb/vt for the MLP phase

    # =========================================================
    # Phase E: proj (fp8 DoubleRow) + residual, LN2, MLP
    # =========================================================
    mlp_pool = tc.tile_pool(name="mlp_pool", bufs=1)
    psC_cm = tc.tile_pool(name="psC", bufs=6, space="PSUM")
    psC = psC_cm.__enter__()
    with mlp_pool as mp:
        wp_sb = mp.tile([128, NCT, C], FP8, name="wp_sb")
        d = nc.sync.dma_start(out=wp_sb, in_=wp8[:, :, :])
        _delay_after(gate_anchor[0], d)
        xt32_sb = []
        for c in range(NCT):
            tl = mp.tile([128, S], F32, name=f"xt32_{c}")
            d = nc.sync.dma_start(out=tl, in_=xt32[c * 128:(c + 1) * 128, :])
            _delay_after(gate_anchor[0], d)
            xt32_sb.append(tl)
        yta8 = mp.tile([128, NCT, S], FP8, name="yta8")
        for hp in range(NCT):
            nc.sync.dma_start(out=yta8[:, hp, :], in_=cc3_out[hp])

        x2t_sb, x2bf_sb = [], []
        for co in range(NCT):
            ps = psC.tile([128, 512], F32, name=f"prps_{co}", tag="ps")
            for dd in range(NDR):
                nc.tensor.matmul(ps,
                                 wp_sb[:, 2 * dd:2 * dd + 2,
                                       co * 128:(co + 1) * 128],
                                 yta8[:, 2 * dd:2 * dd + 2, :],
                                 start=(dd == 0), stop=(dd == NDR - 1),
                                 perf_mode=DR)
            x2 = x2t_p.tile([128, S], F32, name=f"x2t_{co}")
            nc.vector.scalar_tensor_tensor(
                out=x2, in0=ps[:, :], scalar=kproj_b[:, 0:1],
                in1=xt32_sb[co], op0=ALU.mult, op1=ALU.add)
            if use_bproj:
                nc.vector.tensor_scalar(
                    out=x2, in0=x2, scalar1=bproj_sb[:, co:co + 1],
                    scalar2=None, op0=ALU.add)
            x2b = x2t_p.tile([128, S], BF16, name=f"x2bf_{co}")
            nc.vector.tensor_copy(x2b, x2)
            x2t_sb.append(x2)
            x2bf_sb.append(x2b)
            if debug:
                nc.sync.dma_start(out=dbg["d_x2"][co * 128:(co + 1) * 128, :],
                                  in_=x2)

        # ---- LN2 (always unit weight/bias: ln2 w,b folded into fc) ----
        def bcast(tag, src_bf, n):
            ps = psC.tile([128, 512], F32, name=f"{tag}_bc", tag="ps")
            nc.tensor.matmul(ps[:, :n], ones_row[:, :], src_bf[:, :n],
                             start=True, stop=True)
            return ps

        s2_ps = psC.tile([1, 512], F32, name="ln2_sps", tag="st2", bufs=2)
        q2_ps = psC.tile([1, 512], F32, name="ln2_qps", tag="st2", bufs=2)
        for c in range(NCT):
            sq = mp.tile([128, S], BF16, name=f"ln2_sq_{c}", tag="ln2_sq",
                         bufs=3)
            nc.vector.tensor_mul(sq, x2bf_sb[c], x2bf_sb[c])
            nc.tensor.matmul(s2_ps[:, :], ones_col[:, :], x2bf_sb[c][:, :],
                             start=(c == 0), stop=(c == NCT - 1))
            nc.tensor.matmul(q2_ps[:, :], ones_col[:, :], sq[:, :],
                             start=(c == 0), stop=(c == NCT - 1))
        mu_2 = mp.tile([1, S], F32, name="ln2_mu")
        nc.scalar.mul(mu_2, s2_ps[:, :], 1.0 / C)
        msq_2 = mp.tile([1, S], F32, name="ln2_msq")
        nc.scalar.mul(msq_2, q2_ps[:, :], 1.0 / C)
        mu2_2 = mp.tile([1, S], F32, name="ln2_mu2")
        nc.vector.tensor_mul(mu2_2, mu_2, mu_2)
        var_2 = mp.tile([1, S], F32, name="ln2_var")
        nc.vector.tensor_sub(var_2, msq_2, mu2_2)
        lnv_2 = mp.tile([1, S], F32, name="ln2_lnv")
        nc.scalar.activation(lnv_2, var_2, AF.Ln, bias=eps_t, scale=1.0)
        rstd_2 = mp.tile([1, S], F32, name="ln2_rstd")
        nc.scalar.activation(rstd_2, lnv_2, AF.Exp, scale=-0.5)
        rstd2_bf = mp.tile([1, S], BF16, name="ln2_rstd_bf")
        nc.vector.tensor_copy(rstd2_bf, rstd_2)
        nmurs_2 = mp.tile([1, S], F32, name="ln2_nmurs")
        nc.vector.tensor_mul(nmurs_2, mu_2, rstd_2)
        nmurs2_bf = mp.tile([1, S], BF16, name="ln2_nmurs_bf")
        nc.scalar.mul(nmurs2_bf, nmurs_2, -1.0)
        r_ps = bcast("ln2_r", rstd2_bf, S)
        sh_ps = bcast("ln2_sh", nmurs2_bf, S)
        r_b = mp.tile([128, S], BF16, name="ln2_r_b")
        nc.scalar.copy(r_b, r_ps[:, :S])
        sh_b = mp.tile([128, S], BF16, name="ln2_sh_b")
        nc.scalar.copy(sh_b, sh_ps[:, :S])
        ln2t = []
        for c in range(NCT):
            tmp = mp.tile([128, S], BF16, name=f"ln2_tmp_{c}", tag="ln2_tmp",
                          bufs=3)
            nc.vector.tensor_mul(tmp, x2bf_sb[c], r_b)
            o = mp.tile([128, S], BF16, name=f"ln2_o_{c}")
            nc.vector.tensor_add(o, tmp, sh_b)
            ln2t.append(o)
            if debug:
                nc.sync.dma_start(out=dbg["d_ln2"][c * 128:(c + 1) * 128, :],
                                  in_=o)

        # ---- fc + GELU ----
        fw_sb = {}
        for half in range(2):
            for c in range(NCT):
                tl = mp.tile([128, FF // 2], BF16, name=f"fw_{half}_{c}",
                             tag="fw", bufs=10)
                d = nc.sync.dma_start(
                    out=tl,
                    in_=w_fc[c * 128:(c + 1) * 128,
                             half * (FF // 2):(half + 1) * (FF // 2)])
                _delay_after(gate_anchor[0] if half == 0 else a2a_y, d)
                fw_sb[(half, c)] = tl
        ht = []
        for f in range(NFT):
            half, fo = f // (NFT // 2), f % (NFT // 2)
            ps = psC.tile([128, 512], F32, name=f"fcps_{f}", tag="ps")
            for c in range(NCT):
                nc.tensor.matmul(ps[:, :],
                                 fw_sb[(half, c)][:, fo * 128:(fo + 1) * 128],
                                 ln2t[c][:, :],
                                 start=(c == 0), stop=(c == NCT - 1))
            h = mp.tile([128, S], BF16, name=f"ht_{f}")
            if use_bfc:
                nc.scalar.activation(h, ps[:, :], AF.Gelu,
                                     bias=bfc_sb[:, f:f + 1], scale=1.0)
            else:
                nc.scalar.activation(h, ps[:, :], AF.Gelu, scale=1.0)
            ht.append(h)

        # ---- mlp proj + residual -> out ----
        psC_cm.__exit__(None, None, None)
        psM_cm = tc.tile_pool(name="psM", bufs=8, space="PSUM")
        psM = psM_cm.__enter__()
        accs = [psM.tile([128, 512], F32, name=f"mlps_{co}", tag="psm",
                         bufs=8) for co in range(NCT)]
        for f in range(NFT):
            tl = mp.tile([128, C], BF16, name=f"mw_{f}", tag="mw", bufs=3)
            d = nc.sync.dma_start(out=tl, in_=w_mlp[f * 128:(f + 1) * 128, :])
            _delay_after(a2a_y, d)
            for co in range(NCT):
                nc.tensor.matmul(accs[co][:, :],
                                 tl[:, co * 128:(co + 1) * 128],
                                 ht[f][:, :],
                                 start=(f == 0), stop=(f == NFT - 1))
        for co in range(NCT):
            o = mp.tile([128, S], F32, name=f"out_{co}", tag="outt", bufs=3)
            if use_bmlp:
                nc.vector.scalar_tensor_tensor(
                    out=o, in0=accs[co][:, :], scalar=bmlp_sb[:, co:co + 1],
                    in1=x2t_sb[co], op0=ALU.add, op1=ALU.add)
            else:
                nc.vector.tensor_add(o, accs[co][:, :], x2t_sb[co])
            nc.sync.dma_start(out=out_d[co * 128:(co + 1) * 128, :], in_=o)
        psM_cm.__exit__(None, None, None)

    es.close()


# =============================================================
# Host side
# =============================================================
_CACHE = {}


def _get_nc(flags):
    if flags not in _CACHE:
        _CACHE[flags] = build(flags)
    return _CACHE[flags]


def _q8(a, scale):
    return np.clip(np.asarray(a, np.float32) * scale,
                   -240.0, 240.0).astype(NP_FP8)


def _prep(inputs, debug=False):
    f32 = np.float32
    x = np.asarray(inputs["x"], f32)
    attn_w = np.asarray(inputs["attn_w"], f32)
    attn_b = np.asarray(inputs["attn_b"], f32)
    proj_w = np.asarray(inputs["proj_w"], f32)
    proj_b = np.asarray(inputs["proj_b"], f32)
    fc_w = np.asarray(inputs["fc_w"], f32)
    fc_b = np.asarray(inputs["fc_b"], f32)
    mlp_w = np.asarray(inputs["mlp_proj_w"], f32)
    mlp_b = np.asarray(inputs["mlp_proj_b"], f32)
    ln1w = np.asarray(inputs["ln1_w"], f32)
    ln1b = np.asarray(inputs["ln1_b"], f32)
    ln2w = np.asarray(inputs["ln2_w"], f32)
    ln2b = np.asarray(inputs["ln2_b"], f32)

    # fold layernorm affine params into the adjacent matmuls
    w_eff = ln1w[:, None] * attn_w               # [C, 3C]
    b_eff = attn_b + ln1b @ attn_w               # [3C]
    fcw_eff = ln2w[:, None] * fc_w               # [C, FF]
    fcb_eff = fc_b + ln2b @ fc_w                 # [FF]

    def nz(a):
        return bool(np.any(a != 0.0))

    use_bqkv = nz(b_eff)
    flags = (use_bqkv, nz(proj_b), nz(fcb_eff), nz(mlp_b), debug)

    def colsplit(v):
        return np.ascontiguousarray(v.reshape(-1, 128).T)

    bf = lambda a: np.ascontiguousarray(a).astype(NP_BF16)

    x_all = x.reshape(TT, C)
    Sx = 240.0 / max(1e-30, np.abs(x_all).max())
    xq_full = np.ascontiguousarray(
        _q8(x_all.T, Sx).reshape(NCT, 128, TT).transpose(1, 0, 2))

    Swp = 240.0 / max(1e-30, np.abs(proj_w).max())
    wp8_a = np.ascontiguousarray(
        _q8(proj_w, Swp).reshape(NCT, 128, C).transpose(1, 0, 2))

    k_idx = np.arange(128)[:, None]
    q_idx = np.arange(QCH)[None, :]
    m0 = (q_idx >= k_idx).astype(NP_BF16)
    m1 = (q_idx >= k_idx + 128).astype(NP_BF16)
    one = np.ones((128, QCH), NP_BF16)
    shared = {
        "w_fc": bf(fcw_eff), "w_mlp": bf(mlp_w),
        "b_proj": colsplit(proj_b), "b_fc": colsplit(fcb_eff),
        "b_mlp": colsplit(mlp_b),
        "wp8": wp8_a,
        "mask0": np.concatenate([m0, one, m0, one], axis=1),
        "mask1": np.concatenate([m1, one, m1, one], axis=1),
        "mask0x": np.concatenate([m0, m0], axis=1),
        "mask1x": np.concatenate([m1, m1], axis=1),
        "xq": xq_full,
    }

    in_maps = []
    for i in range(CORES):
        b, s = i // 4, i % 4
        xs = np.ascontiguousarray(x[b, s * S:(s + 1) * S, :].T)  # [C, S]
        # my heads' qkv weight columns: q, k, v blocks of 128 cols each
        cols = np.concatenate([np.arange(128) + 128 * i + blk * C
                               for blk in range(3)])
        wsl = w_eff[:, cols]                      # [C, 384]
        Sw = 240.0 / max(1e-30, np.abs(wsl).max())
        w8 = _q8(wsl, Sw)
        w8f = w8.astype(f32) / Sw
        wsum_deq = w8f.sum(axis=0)                # [384]
        bsl = b_eff[cols]
        m = dict(shared)
        m["xt"] = xs.astype(NP_BF16)
        m["xt32"] = xs
        m["wq8"] = np.ascontiguousarray(
            w8.reshape(NCT, 128, 384).transpose(1, 0, 2))
        m["wsum3"] = np.ascontiguousarray(wsum_deq.reshape(3, 128).T)
        m["wvsum"] = np.ascontiguousarray(wsum_deq[256:384].reshape(1, 128))
        m["b_qkv"] = np.ascontiguousarray(bsl.reshape(3, 128).T)
        m["bv_row"] = np.ascontiguousarray(bsl[256:384].reshape(1, 128))
        m["scales"] = np.array(
            [[1.0 / (Sx * Sw), 1.0 / (SY * Swp), 0, 0, 0, 0, 0, 0]], f32)
        in_maps.append(m)
    return flags, in_maps


def run_sharded(inputs, debug=False, trace=False, trace_kwargs=None):
    flags, in_maps = _prep(inputs, debug)
    nc = _get_nc(flags)
    res = bass_utils.run_bass_kernel_spmd(
        nc, in_maps, core_ids=list(range(CORES)), trace=trace,
        **(trace_kwargs or {}))
    out = np.empty((B, T, C), np.float32)
    for i in range(CORES):
        b, s = i // 4, i % 4
        out[b, s * S:(s + 1) * S, :] = res.results[i]["out"].T
    return out, res


def kernel(**inputs):
    out, _ = run_sharded(inputs, debug=False, trace=False)
    return out


# revision 23
# speedup vs baseline: 1.0564x; 1.0564x over previous
"""Trainium2 8-core kernel for a GPT-style transformer block (v3).

Strategy:
  - Megatron-style QKV: every core holds the FULL (quantized) x and the QKV
    weight columns of its own 2 heads, so Q^T/K^T/V for all 4096 tokens are
    computed locally -- no pre-attention collectives at all.
  - LN1 is folded algebraically into the QKV matmul: qkv = rstd*(x@W) -
    mu*rstd*colsum(W). Per-token stats (sum x, sum x^2) are computed on each
    core's own 512-token span and shared with one tiny (4KB) AllGather.
  - QKV and proj matmuls run in fp8 (e4m3) DoubleRow mode: K=256 per
    instruction, 2x PE throughput. fc/mlp stay bf16 (fp8 fails tolerance).
  - Attention (2 heads/core, all tokens) processes query-chunk pairs with
    shared key tiles; softmax denominators ride along as a ones-column in V;
    normalization uses vector reciprocal (no scalar table switches).
  - y returns token-sharded via one fp8 AllToAll (0.5MB); proj + residual +
    LN2 + MLP are local to the core's 512 tokens.
"""

import sys

sys.path.insert(0, "/opt/trn_rl_repo")

import numpy as np
import ml_dtypes

import concourse.bass as bass
import concourse.mybir as mybir
import concourse.tile as tile
from concourse import bacc, bass_utils

BF16 = mybir.dt.bfloat16
F32 = mybir.dt.float32
FP8 = mybir.dt.float8e4
AF = mybir.ActivationFunctionType
ALU = mybir.AluOpType
DR = mybir.MatmulPerfMode.DoubleRow
NP_BF16 = ml_dtypes.bfloat16
NP_FP8 = ml_dtypes.float8_e4m3

B, T, C, H, HS, FF = 2, 2048, 1024, 16, 64, 4096
CORES = 8
S = 512            # tokens per core (MLP/residual shard)
TT = B * T         # 4096 total tokens
NCT = C // 128     # 8 feature tiles
NDR = NCT // 2     # 4 DoubleRow contraction pairs
NFT = FF // 128    # 32 mlp hidden tiles
QCH = 256          # query chunk
NQC = T // QCH     # 8 query chunks per batch
NKT = T // 128     # 16 key tiles per batch
NTT = TT // 128    # 32 token tiles total
EPS = 1e-5
SY = 30.0          # fixed y quantization scale (|y| < 8 by softmax convexity)


def build(flags):
    (use_bqkv, use_bproj, use_bfc, use_bmlp, debug) = flags

    nc = bacc.Bacc("TRN2", target_bir_lowering=False, debug=False,
                   num_devices=CORES)

    # ---------------- DRAM inputs ----------------
    xq = nc.dram_tensor("xq", [128, NCT, TT], FP8, kind="ExternalInput")
    xt32 = nc.dram_tensor("xt32", [C, S], F32, kind="ExternalInput")
    wq8 = nc.dram_tensor("wq8", [128, NCT, 384], FP8, kind="ExternalInput")
    wsum3 = nc.dram_tensor("wsum3", [128, 3], F32, kind="ExternalInput")
    wvsum = nc.dram_tensor("wvsum", [1, 128], F32, kind="ExternalInput")
    wp8 = nc.dram_tensor("wp8", [128, NCT, C], FP8, kind="ExternalInput")
    scales = nc.dram_tensor("scales", [1, 8], F32, kind="ExternalInput")
    b_qkv = nc.dram_tensor("b_qkv", [128, 3], F32, kind="ExternalInput")
    bv_row = nc.dram_tensor("bv_row", [1, 128], F32, kind="ExternalInput")
    w_fc = nc.dram_tensor("w_fc", [C, FF], BF16, kind="ExternalInput")
    w_mlp = nc.dram_tensor("w_mlp", [FF, C], BF16, kind="ExternalInput")
    b_proj = nc.dram_tensor("b_proj", [128, NCT], F32, kind="ExternalInput")
    b_fc = nc.dram_tensor("b_fc", [128, NFT], F32, kind="ExternalInput")
    b_mlp = nc.dram_tensor("b_mlp", [128, NCT], F32, kind="ExternalInput")
    mask0_d = nc.dram_tensor("mask0", [128, 4 * QCH], BF16, kind="ExternalInput")
    mask1_d = nc.dram_tensor("mask1", [128, 4 * QCH], BF16, kind="ExternalInput")
    mask0x_d = nc.dram_tensor("mask0x", [128, 2 * QCH], BF16, kind="ExternalInput")
    mask1x_d = nc.dram_tensor("mask1x", [128, 2 * QCH], BF16, kind="ExternalInput")
    out_d = nc.dram_tensor("out", [C, S], F32, kind="ExternalOutput")
    dbg = {}
    if debug:
        for nm, shp, dt in [("d_qt", [128, TT], BF16), ("d_kt", [128, TT], BF16),
                            ("d_v", [TT, 128], BF16), ("d_yt", [128, TT], FP8),
                            ("d_x2", [C, S], F32), ("d_ln2", [C, S], BF16)]:
            dbg[nm] = nc.dram_tensor(nm, shp, dt, kind="ExternalOutput")

    with tile.TileContext(nc) as tc:
        _build_body(nc, tc, locals(), flags)
    nc.compile()
    return nc


def _build_body(nc, tc, t_, flags):
    (use_bqkv, use_bproj, use_bfc, use_bmlp, debug) = flags
    xq, xt32 = t_["xq"], t_["xt32"]
    wq8, wsum3, wvsum, wp8 = t_["wq8"], t_["wsum3"], t_["wvsum"], t_["wp8"]
    scales, b_qkv, bv_row = t_["scales"], t_["b_qkv"], t_["bv_row"]
    w_fc, w_mlp = t_["w_fc"], t_["w_mlp"]
    b_proj, b_fc, b_mlp = t_["b_proj"], t_["b_fc"], t_["b_mlp"]
    mask0_d, mask1_d = t_["mask0_d"], t_["mask1_d"]
    mask0x_d, mask1x_d = t_["mask0x_d"], t_["mask1x_d"]
    out_d, dbg = t_["out_d"], t_["dbg"]

    from contextlib import ExitStack
    from concourse.tile import add_dep_helper

    def _delay_after(frm, to):
        f = frm.ins if hasattr(frm, "ins") else frm
        t = to.ins if hasattr(to, "ins") else to
        add_dep_helper(t, f, sync=True, reason="delay heavy DMA")
    es = ExitStack()

    consts = es.enter_context(tc.tile_pool(name="consts", bufs=1))
    dram = es.enter_context(tc.tile_pool(name="dram", bufs=1, space="DRAM"))
    # q/k/v activation tiles live from QKV production through attention
    qkvt_cm = tc.tile_pool(name="qkvt_p", bufs=1, side="right")
    qkvt_p = qkvt_cm.__enter__()  # closed manually after attention
    x2t_p = es.enter_context(tc.tile_pool(name="x2t_p", bufs=1))

    # ---- constants ----
    ones_col = consts.tile([128, 1], BF16, name="ones_col")
    nc.vector.memset(ones_col, 1.0)
    ones_row = consts.tile([1, 128], BF16, name="ones_row")
    nc.vector.memset(ones_row, 1.0)
    eps_t = consts.tile([1, 1], F32, name="eps_t")
    nc.vector.memset(eps_t, EPS)
    mask0 = consts.tile([128, 4 * QCH], BF16, name="mask0")
    nc.sync.dma_start(out=mask0, in_=mask0_d[:, :])
    mask1 = consts.tile([128, 4 * QCH], BF16, name="mask1")
    nc.sync.dma_start(out=mask1, in_=mask1_d[:, :])
    mask0x = consts.tile([128, 2 * QCH], BF16, name="mask0x")
    nc.sync.dma_start(out=mask0x, in_=mask0x_d[:, :])
    mask1x = consts.tile([128, 2 * QCH], BF16, name="mask1x")
    nc.sync.dma_start(out=mask1x, in_=mask1x_d[:, :])
    sc_sb = consts.tile([1, 8], F32, name="sc_sb")
    nc.sync.dma_start(out=sc_sb, in_=scales[:, :])
    wsum_sb = consts.tile([128, 3], F32, name="wsum_sb")
    nc.sync.dma_start(out=wsum_sb, in_=wsum3[:, :])
    wvsum_sb = consts.tile([1, 128], F32, name="wvsum_sb")
    nc.sync.dma_start(out=wvsum_sb, in_=wvsum[:, :])
    wvsum_b = consts.tile([128, 128], F32, name="wvsum_b")
    nc.gpsimd.partition_broadcast(wvsum_b, wvsum_sb)
    kproj_b = consts.tile([128, 1], F32, name="kproj_b")
    nc.gpsimd.partition_broadcast(kproj_b, sc_sb[0:1, 1:2])
    if use_bqkv:
        bqkv_sb = consts.tile([128, 3], F32, name="bqkv_sb")
        nc.sync.dma_start(out=bqkv_sb, in_=b_qkv[:, :])
        bv_sb = consts.tile([1, 128], F32, name="bv_sb")
        nc.sync.dma_start(out=bv_sb, in_=bv_row[:, :])
        bv_b = consts.tile([128, 128], F32, name="bv_b")
        nc.gpsimd.partition_broadcast(bv_b, bv_sb)
    bproj_sb = (consts.tile([128, NCT], F32, name="bproj_sb")
                if use_bproj else None)
    if use_bproj:
        nc.sync.dma_start(out=bproj_sb, in_=b_proj[:, :])
    bfc_sb = consts.tile([128, NFT], F32, name="bfc_sb") if use_bfc else None
    if use_bfc:
        nc.sync.dma_start(out=bfc_sb, in_=b_fc[:, :])
    bmlp_sb = consts.tile([128, NCT], F32, name="bmlp_sb") if use_bmlp else None
    if use_bmlp:
        nc.sync.dma_start(out=bmlp_sb, in_=b_mlp[:, :])

    # ---- collective DRAM tiles ----
    cc0_in = dram.tile([1, 128], BF16, name="cc0_in")
    cc0_out = dram.tile([CORES, 128], BF16, name="cc0_out")
    rr0 = dram.tile([1, TT], F32, name="rr0")
    rr1 = dram.tile([1, TT], F32, name="rr1")
    rr0b = dram.tile([1, TT], BF16, name="rr0b")
    rr1b = dram.tile([1, TT], BF16, name="rr1b")
    cc3_in = dram.tile([CORES, 128, S], FP8, name="cc3_in")
    cc3_out = dram.tile([CORES, 128, S], FP8, name="cc3_out")

    # comms warmup
    nc.sync.dma_start(out=cc0_in, in_=mask0_d[0:1, 0:128])
    nc.gpsimd.collective_compute(
        "AllGather", ALU.bypass,
        replica_groups=[list(range(CORES))],
        ins=[cc0_in[:, :].opt()],
        outs=[cc0_out[:, :].opt()])

    # persistent attention inputs
    qtb = [qkvt_p.tile([128, T], BF16, name=f"qtb_{b}") for b in range(B)]
    ktb = [qkvt_p.tile([128, T], BF16, name=f"ktb_{b}") for b in range(B)]
    vt = [qkvt_p.tile([128, 2, 128], BF16, name=f"vt_{tt}")
          for tt in range(NTT)]
    for v3 in vt:
        nc.gpsimd.memset(v3[:, :, 0:1], 1.0)
        nc.gpsimd.memset(v3[:, :, 1:64], 0.0)

    # =========================================================
    # Phase B: LN1 statistics on my span + AllGather
    # =========================================================
    qkv_cm = tc.tile_pool(name="qkv_pool", bufs=1)
    qp = qkv_cm.__enter__()
    psQ_cm = tc.tile_pool(name="psQ", bufs=1, space="PSUM")
    psQ = psQ_cm.__enter__()

    # full-token LN1 stats computed locally per chunk (no collectives on the
    # critical path: the first collective of an execution costs ~95us)
    ones8 = consts.tile([128, 2, 16], FP8, name="ones8")
    nc.vector.memset(ones8, 1.0)
    rqk_row = qp.tile([1, TT], BF16, name="rqk_row")
    nm_row = qp.tile([1, TT], BF16, name="nm_row")
    rqk_row32 = qp.tile([1, TT], F32, name="rqk_row32")
    nm_row32 = qp.tile([1, TT], F32, name="nm_row32")
    rcol = qp.tile([128, NTT], F32, name="rcol")
    ncol = qp.tile([128, NTT], F32, name="ncol")

    # =========================================================
    # Phase C: QKV in fp8 DoubleRow
    # =========================================================
    wq_sb = qp.tile([128, NCT, 384], FP8, name="wq_sb")
    nc.sync.dma_start(out=wq_sb, in_=wq8[:, :, :])

    for ch in range(NCT):
        cols = slice(ch * 512, (ch + 1) * 512)
        bi, r4 = ch // 4, ch % 4
        xc = qp.tile([128, NCT, 512], FP8, name=f"xc_{ch}", tag="xc", bufs=3)
        nc.sync.dma_start(out=xc, in_=xq[:, :, cols])
        # ---- per-chunk token stats ----
        s_ps = psQ.tile([16, 512], F32, name=f"sps_{ch}", tag="st", bufs=2)
        for d in range(NDR):
            nc.tensor.matmul(s_ps, ones8[:, :, :], xc[:, 2 * d:2 * d + 2, :],
                             start=(d == 0), stop=(d == NDR - 1),
                             perf_mode=DR)
        q_ps = psQ.tile([16, 512], F32, name=f"qps_{ch}", tag="st", bufs=2)
        for d in range(NDR):
            sq8t = qp.tile([128, 2, 512], FP8, name=f"sq_{ch}_{d}", tag="sq",
                           bufs=3)
            nc.scalar.activation(sq8t, xc[:, 2 * d:2 * d + 2, :], AF.Square,
                                 scale=1.0 / 16.0)
            nc.tensor.matmul(q_ps, ones8[:, :, :], sq8t[:, :, :],
                             start=(d == 0), stop=(d == NDR - 1),
                             perf_mode=DR)
        mu = qp.tile([1, 512], F32, name=f"mu_{ch}", tag="mu", bufs=2)
        nc.scalar.mul(mu, s_ps[0:1, :], sc_sb[0:1, 2:3])
        msq = qp.tile([1, 512], F32, name=f"msq_{ch}", tag="msq", bufs=2)
        nc.scalar.mul(msq, q_ps[0:1, :], sc_sb[0:1, 3:4])
        var = qp.tile([1, 512], F32, name=f"var_{ch}", tag="var", bufs=2)
        nc.vector.tensor_mul(var, mu, mu)
        nc.vector.tensor_sub(var, msq, var)
        lnv = qp.tile([1, 512], F32, name=f"lnv_{ch}", tag="lnv", bufs=2)
        nc.scalar.activation(lnv, var, AF.Ln, bias=eps_t, scale=1.0)
        rstd = qp.tile([1, 512], F32, name=f"rstd_{ch}", tag="rstd", bufs=2)
        nc.scalar.activation(rstd, lnv, AF.Exp, scale=-0.5)
        rqk32_sl = rqk_row32[0:1, cols]
        nc.vector.tensor_scalar(out=rqk32_sl, in0=rstd,
                                scalar1=sc_sb[0:1, 0:1], scalar2=None,
                                op0=ALU.mult)
        nc.vector.tensor_copy(rqk_row[0:1, cols], rqk32_sl)
        nm32_sl = nm_row32[0:1, cols]
        nc.vector.tensor_mul(nm32_sl, mu, rstd)
        nc.scalar.mul(nm32_sl, nm32_sl, -1.0)
        nc.vector.tensor_copy(nm_row[0:1, cols], nm32_sl)
        nc.sync.dma_start(out=rr0[0:1, cols], in_=rqk32_sl)
        nc.sync.dma_start(out=rr1[0:1, cols], in_=nm32_sl)
        rca = rr0.rearrange("o (j p) -> (o p) j", p=128)
        nca = rr1.rearrange("o (j p) -> (o p) j", p=128)
        nc.sync.dma_start(out=rcol[:, ch * 4:(ch + 1) * 4],
                          in_=rca[:, ch * 4:(ch + 1) * 4])
        nc.sync.dma_start(out=ncol[:, ch * 4:(ch + 1) * 4],
                          in_=nca[:, ch * 4:(ch + 1) * 4])
        rqk_b = qp.tile([128, 512], BF16, name=f"rqkb_{ch}", tag="rqkb",
                        bufs=2)
        nc.gpsimd.partition_broadcast(rqk_b, rqk_row[0:1, cols])
        nm_b = qp.tile([128, 512], BF16, name=f"nmb_{ch}", tag="nmb", bufs=2)
        nc.gpsimd.partition_broadcast(nm_b, nm_row[0:1, cols])
        for o in range(2):  # 0=q, 1=k
            ps = psQ.tile([128, 512], F32, name=f"qk_{ch}_{o}", tag="qk",
                          bufs=3)
            for d in range(NDR):
                nc.tensor.matmul(ps,
                                 wq_sb[:, 2 * d:2 * d + 2,
                                       o * 128:(o + 1) * 128],
                                 xc[:, 2 * d:2 * d + 2, :],
                                 start=(d == 0), stop=(d == NDR - 1),
                                 perf_mode=DR)
            tmp = qp.tile([128, 512], BF16, name=f"qkt_{ch}_{o}", tag="qkt",
                          bufs=3)
            nc.vector.tensor_mul(tmp, ps, rqk_b)
            dst = (qtb if o == 0 else ktb)[bi][:, r4 * 512:(r4 + 1) * 512]
            nc.vector.scalar_tensor_tensor(
                out=dst, in0=nm_b, scalar=wsum_sb[:, o:o + 1],
                in1=tmp, op0=ALU.mult, op1=ALU.add)
            if use_bqkv:
                nc.vector.tensor_scalar(
                    out=dst, in0=dst, scalar1=bqkv_sb[:, o:o + 1],
                    scalar2=None, op0=ALU.add)
        for t4 in range(4):
            ttile = ch * 4 + t4
            tc_sl = slice(t4 * 128, (t4 + 1) * 128)
            psv = psQ.tile([128, 128], F32, name=f"vps_{ttile}", tag="vps",
                           bufs=3)
            for d in range(NDR):
                nc.tensor.matmul(psv,
                                 xc[:, 2 * d:2 * d + 2, tc_sl],
                                 wq_sb[:, 2 * d:2 * d + 2, 256:384],
                                 start=(d == 0), stop=(d == NDR - 1),
                                 perf_mode=DR)
            outer = qp.tile([128, 128], F32, name=f"outer_{ttile}",
                            tag="outer", bufs=3)
            nc.vector.tensor_scalar(
                out=outer, in0=wvsum_b, scalar1=ncol[:, ttile:ttile + 1],
                scalar2=None, op0=ALU.mult)
            if use_bqkv:
                nc.vector.tensor_add(outer, outer, bv_b)
            v3 = vt[ttile]
            for h in range(2):
                nc.vector.scalar_tensor_tensor(
                    out=v3[:, h, 64:128], in0=psv[:, h * 64:(h + 1) * 64],
                    scalar=rcol[:, ttile:ttile + 1], in1=outer[:, h * 64:(h + 1) * 64],
                    op0=ALU.mult, op1=ALU.add)
        if debug:
            for o, nm in ((0, "d_qt"), (1, "d_kt")):
                nc.sync.dma_start(
                    out=dbg[nm][:, ch * 512:(ch + 1) * 512],
                    in_=(qtb if o == 0 else ktb)[bi][:, r4 * 512:(r4 + 1) * 512])
    if debug:
        for ttile in range(NTT):
            for h in range(2):
                nc.sync.dma_start(
                    out=dbg["d_v"][ttile * 128:(ttile + 1) * 128,
                                   h * 64:(h + 1) * 64],
                    in_=vt[ttile][:, h, 64:128])
    psQ_cm.__exit__(None, None, None)
    qkv_cm.__exit__(None, None, None)

    # =========================================================
    # Phase D: attention (my 2 heads, all tokens)
    # =========================================================
    att_pool = tc.tile_pool(name="att_pool", bufs=1)
    psB_pool = tc.tile_pool(name="psB", bufs=2, space="PSUM")
    W2 = 2 * QCH
    gate_anchor = [None]
    a2a_y = None
    with att_pool as ap, psB_pool as psB:

        def flush_normalize(item):
            fb, fp, y_A, y_B = item
            j = 4 * fb + fp
            for hi, y_ps in ((0, y_A), (1, y_B)):
                # copy out of PSUM immediately so the accumulator bank frees
                dn = ap.tile([1, W2], F32, name=f"dn_{fb}_{fp}_{hi}",
                             tag="dn", bufs=4)
                nc.vector.tensor_copy(dn, y_ps[0:1, :])
                ytb = ap.tile([128, W2], BF16, name=f"ytb_{fb}_{fp}_{hi}",
                              tag="ytb", bufs=4)
                nc.vector.tensor_copy(ytb[64:128, :], y_ps[64:128, :])
                rec = ap.tile([1, W2], F32, name=f"rec_{fb}_{fp}_{hi}",
                              tag="rec", bufs=4)
                nc.vector.reciprocal(rec, dn)
                rec_bf = ap.tile([1, W2], BF16, name=f"recbf_{fb}_{fp}_{hi}",
                                 tag="recbf", bufs=4)
                nc.scalar.mul(rec_bf, rec, SY)
                den = ap.tile([128, W2], BF16, name=f"den_{fb}_{fp}_{hi}",
                              tag="den", bufs=4)
                nc.gpsimd.partition_broadcast(den, rec_bf)
                yt8 = ap.tile([128, W2], FP8, name=f"yt8_{fb}_{fp}_{hi}",
                              tag="yt8", bufs=4)
                nc.vector.tensor_mul(yt8[64:128, :], ytb[64:128, :],
                                     den[64:128, :])
                nc.sync.dma_start(out=cc3_in[j, hi * 64:(hi + 1) * 64, :],
                                  in_=yt8[64:128, :])

        for b in range(B):
            for p in reversed(range(NQC // 2)):
                qc = 2 * p
                qs = qc * QCH
                nsh = 2 * (qc + 1)
                y_A = psB.tile([128, W2], F32, name=f"yA_{b}_{p}", tag="ya",
                               bufs=2)
                y_B = psB.tile([128, W2], F32, name=f"yB_{b}_{p}", tag="yb",
                               bufs=2)
                for kt in range(nsh + 2):
                    shared = kt < nsh
                    cols = slice(0, W2) if shared else slice(QCH, W2)
                    ncols = W2 if shared else QCH
                    s_AB = psB.tile([128, 2 * W2], F32, name=f"s_{b}_{p}_{kt}",
                                    tag="ps2", bufs=2)
                    nc.tensor.matmul(s_AB[:, 0:ncols],
                                     ktb[b][0:64, kt * 128:(kt + 1) * 128],
                                     qtb[b][0:64, qs + cols.start:qs + W2],
                                     start=True, stop=True)
                    nc.tensor.matmul(s_AB[:, W2:W2 + ncols],
                                     ktb[b][64:128, kt * 128:(kt + 1) * 128],
                                     qtb[b][64:128, qs + cols.start:qs + W2],
                                     start=True, stop=True)
                    e_AB = ap.tile([128, 2 * W2], BF16, name=f"e_{b}_{p}_{kt}",
                                   tag="eAB", bufs=5)
                    if shared:
                        nc.scalar.activation(e_AB, s_AB[:, :], AF.Exp,
                                             scale=1.0 / np.sqrt(HS))
                        if kt in (qc * 2, qc * 2 + 1):
                            mx = mask0x if kt == qc * 2 else mask1x
                            e4 = e_AB.rearrange("p (h c q) -> p h c q",
                                                h=2, c=2)
                            m4 = mx.rearrange("p (h q) -> p h q", h=2)
                            nc.vector.tensor_mul(e4[:, :, 0, :],
                                                 e4[:, :, 0, :], m4)
                    else:
                        e3 = e_AB.rearrange("p (h q) -> p h q", h=2)
                        s3 = s_AB.rearrange("p (h q) -> p h q", h=2)
                        nc.scalar.activation(e3[:, :, 0:QCH], s3[:, :, 0:QCH],
                                             AF.Exp, scale=1.0 / np.sqrt(HS))
                        mx = mask0x if kt == nsh else mask1x
                        nc.vector.tensor_mul(
                            e3[:, :, 0:QCH], e3[:, :, 0:QCH],
                            mx.rearrange("p (h q) -> p h q", h=2))
                    v3 = vt[b * NKT + kt]
                    nc.tensor.matmul(y_A[:, cols], v3[:, 0, :],
                                     e_AB[:, 0:ncols],
                                     start=(kt == 0), stop=(kt == nsh + 1),
                                     skip_group_check=True)
                    mmB = nc.tensor.matmul(y_B[:, cols], v3[:, 1, :],
                                           e_AB[:, W2:W2 + ncols],
                                           start=(kt == 0),
                                           stop=(kt == nsh + 1),
                                           skip_group_check=True)
                    if p == 3 and kt == nsh + 1:
                        gate_anchor.append(mmB)
                flush_normalize((b, p, y_A, y_B))
        gate_anchor[0] = gate_anchor[1]
        a2a_y = nc.gpsimd.collective_compute(
            "AllToAll", ALU.bypass,
            replica_groups=[list(range(CORES))],
            ins=[cc3_in[:, :, :].opt()],
            outs=[cc3_out[:, :, :].opt()])
    if debug:
        for j in range(CORES):
            nc.sync.dma_start(out=dbg["d_yt"][:, j * 512:(j + 1) * 512],
                              in_=cc3_in[j, :, :])
    qkvt_cm.__exit__(None, None, None)  # free qtb/ktb/vt for the MLP phase

    # =========================================================
    # Phase E: proj (fp8 DoubleRow) + residual, LN2, MLP
    # =========================================================
    mlp_pool = tc.tile_pool(name="mlp_pool", bufs=1)
    psC_cm = tc.tile_pool(name="psC", bufs=6, space="PSUM")
    psC = psC_cm.__enter__()
    with mlp_pool as mp:
        wp_sb = mp.tile([128, NCT, C], FP8, name="wp_sb")
        d = nc.sync.dma_start(out=wp_sb, in_=wp8[:, :, :])
        _delay_after(gate_anchor[0], d)
        xt32_sb = []
        for c in range(NCT):
            tl = mp.tile([128, S], F32, name=f"xt32_{c}")
            d = nc.sync.dma_start(out=tl, in_=xt32[c * 128:(c + 1) * 128, :])
            _delay_after(gate_anchor[0], d)
            xt32_sb.append(tl)
        yta8 = mp.tile([128, NCT, S], FP8, name="yta8")
        for hp in range(NCT):
            nc.sync.dma_start(out=yta8[:, hp, :], in_=cc3_out[hp])

        x2t_sb, x2bf_sb = [], []
        for co in range(NCT):
            ps = psC.tile([128, 512], F32, name=f"prps_{co}", tag="ps")
            for dd in range(NDR):
                nc.tensor.matmul(ps,
                                 wp_sb[:, 2 * dd:2 * dd + 2,
                                       co * 128:(co + 1) * 128],
                                 yta8[:, 2 * dd:2 * dd + 2, :],
                                 start=(dd == 0), stop=(dd == NDR - 1),
                                 perf_mode=DR)
            x2 = x2t_p.tile([128, S], F32, name=f"x2t_{co}")
            nc.vector.scalar_tensor_tensor(
                out=x2, in0=ps[:, :], scalar=kproj_b[:, 0:1],
                in1=xt32_sb[co], op0=ALU.mult, op1=ALU.add)
            if use_bproj:
                nc.vector.tensor_scalar(
                    out=x2, in0=x2, scalar1=bproj_sb[:, co:co + 1],
                    scalar2=None, op0=ALU.add)
            x2b = x2t_p.tile([128, S], BF16, name=f"x2bf_{co}")
            nc.vector.tensor_copy(x2b, x2)
            x2t_sb.append(x2)
            x2bf_sb.append(x2b)
            if debug:
                nc.sync.dma_start(out=dbg["d_x2"][co * 128:(co + 1) * 128, :],
                                  in_=x2)

        # ---- LN2 (always unit weight/bias: ln2 w,b folded into fc) ----
        def bcast(tag, src_bf, n):
            ps = psC.tile([128, 512], F32, name=f"{tag}_bc", tag="ps")
            nc.tensor.matmul(ps[:, :n], ones_row[:, :], src_bf[:, :n],
                             start=True, stop=True)
            return ps

        s2_ps = psC.tile([1, 512], F32, name="ln2_sps", tag="st2", bufs=2)
        q2_ps = psC.tile([1, 512], F32, name="ln2_qps", tag="st2", bufs=2)
        for c in range(NCT):
            sq = mp.tile([128, S], BF16, name=f"ln2_sq_{c}", tag="ln2_sq",
                         bufs=3)
            nc.vector.tensor_mul(sq, x2bf_sb[c], x2bf_sb[c])
            nc.tensor.matmul(s2_ps[:, :], ones_col[:, :], x2bf_sb[c][:, :],
                             start=(c == 0), stop=(c == NCT - 1))
            nc.tensor.matmul(q2_ps[:, :], ones_col[:, :], sq[:, :],
                             start=(c == 0), stop=(c == NCT - 1))
        mu_2 = mp.tile([1, S], F32, name="ln2_mu")
        nc.scalar.mul(mu_2, s2_ps[:, :], 1.0 / C)
        msq_2 = mp.tile([1, S], F32, name="ln2_msq")
        nc.scalar.mul(msq_2, q2_ps[:, :], 1.0 / C)
        mu2_2 = mp.tile([1, S], F32, name="ln2_mu2")
        nc.vector.tensor_mul(mu2_2, mu_2, mu_2)
        var_2 = mp.tile([1, S], F32, name="ln2_var")
        nc.vector.tensor_sub(var_2, msq_2, mu2_2)
        lnv_2 = mp.tile([1, S], F32, name="ln2_lnv")
        nc.scalar.activation(lnv_2, var_2, AF.Ln, bias=eps_t, scale=1.0)
        rstd_2 = mp.tile([1, S], F32, name="ln2_rstd")
        nc.scalar.activation(rstd_2, lnv_2, AF.Exp, scale=-0.5)
        rstd2_bf = mp.tile([1, S], BF16, name="ln2_rstd_bf")
        nc.vector.tensor_copy(rstd2_bf, rstd_2)
        nmurs_2 = mp.tile([1, S], F32, name="ln2_nmurs")
        nc.vector.tensor_mul(nmurs_2, mu_2, rstd_2)
        nmurs2_bf = mp.tile([1, S], BF16, name="ln2_nmurs_bf")
        nc.scalar.mul(nmurs2_bf, nmurs_2, -1.0)
        r_ps = bcast("ln2_r", rstd2_bf, S)
        sh_ps = bcast("ln2_sh", nmurs2_bf, S)
        r_b = mp.tile([128, S], BF16, name="ln2_r_b")
        nc.scalar.copy(r_b, r_ps[:, :S])
        sh_b = mp.tile([128, S], BF16, name="ln2_sh_b")
        nc.scalar.copy(sh_b, sh_ps[:, :S])
        ln2t = []
        for c in range(NCT):
            tmp = mp.tile([128, S], BF16, name=f"ln2_tmp_{c}", tag="ln2_tmp",
                          bufs=3)
            nc.vector.tensor_mul(tmp, x2bf_sb[c], r_b)
            o = mp.tile([128, S], BF16, name=f"ln2_o_{c}")
            nc.vector.tensor_add(o, tmp, sh_b)
            ln2t.append(o)
            if debug:
                nc.sync.dma_start(out=dbg["d_ln2"][c * 128:(c + 1) * 128, :],
                                  in_=o)

        # ---- fc + GELU ----
        fw_sb = {}
        for half in range(2):
            for c in range(NCT):
                tl = mp.tile([128, FF // 2], BF16, name=f"fw_{half}_{c}",
                             tag="fw", bufs=10)
                d = nc.sync.dma_start(
                    out=tl,
                    in_=w_fc[c * 128:(c + 1) * 128,
                             half * (FF // 2):(half + 1) * (FF // 2)])
                _delay_after(gate_anchor[0] if half == 0 else a2a_y, d)
                fw_sb[(half, c)] = tl
        ht = []
        for f in range(NFT):
            half, fo = f // (NFT // 2), f % (NFT // 2)
            ps = psC.tile([128, 512], F32, name=f"fcps_{f}", tag="ps")
            for c in range(NCT):
                nc.tensor.matmul(ps[:, :],
                                 fw_sb[(half, c)][:, fo * 128:(fo + 1) * 128],
                                 ln2t[c][:, :],
                                 start=(c == 0), stop=(c == NCT - 1))
            h = mp.tile([128, S], BF16, name=f"ht_{f}")
            if use_bfc:
                nc.scalar.activation(h, ps[:, :], AF.Gelu,
                                     bias=bfc_sb[:, f:f + 1], scale=1.0)
            else:
                nc.scalar.activation(h, ps[:, :], AF.Gelu, scale=1.0)
            ht.append(h)

        # ---- mlp proj + residual -> out ----
        psC_cm.__exit__(None, None, None)
        psM_cm = tc.tile_pool(name="psM", bufs=8, space="PSUM")
        psM = psM_cm.__enter__()
        accs = [psM.tile([128, 512], F32, name=f"mlps_{co}", tag="psm",
                         bufs=8) for co in range(NCT)]
        for f in range(NFT):
            tl = mp.tile([128, C], BF16, name=f"mw_{f}", tag="mw", bufs=3)
            d = nc.sync.dma_start(out=tl, in_=w_mlp[f * 128:(f + 1) * 128, :])
            _delay_after(a2a_y, d)
            for co in range(NCT):
                nc.tensor.matmul(accs[co][:, :],
                                 tl[:, co * 128:(co + 1) * 128],
                                 ht[f][:, :],
                                 start=(f == 0), stop=(f == NFT - 1))
        for co in range(NCT):
            o = mp.tile([128, S], F32, name=f"out_{co}", tag="outt", bufs=3)
            if use_bmlp:
                nc.vector.scalar_tensor_tensor(
                    out=o, in0=accs[co][:, :], scalar=bmlp_sb[:, co:co + 1],
                    in1=x2t_sb[co], op0=ALU.add, op1=ALU.add)
            else:
                nc.vector.tensor_add(o, accs[co][:, :], x2t_sb[co])
            nc.sync.dma_start(out=out_d[co * 128:(co + 1) * 128, :], in_=o)
        psM_cm.__exit__(None, None, None)

    es.close()


# =============================================================
# Host side
# =============================================================
_CACHE = {}


def _get_nc(flags):
    if flags not in _CACHE:
        _CACHE[flags] = build(flags)
    return _CACHE[flags]


def _q8(a, scale):
    return np.clip(np.asarray(a, np.float32) * scale,
                   -240.0, 240.0).astype(NP_FP8)


def _prep(inputs, debug=False):
    f32 = np.float32
    x = np.asarray(inputs["x"], f32)
    attn_w = np.asarray(inputs["attn_w"], f32)
    attn_b = np.asarray(inputs["attn_b"], f32)
    proj_w = np.asarray(inputs["proj_w"], f32)
    proj_b = np.asarray(inputs["proj_b"], f32)
    fc_w = np.asarray(inputs["fc_w"], f32)
    fc_b = np.asarray(inputs["fc_b"], f32)
    mlp_w = np.asarray(inputs["mlp_proj_w"], f32)
    mlp_b = np.asarray(inputs["mlp_proj_b"], f32)
    ln1w = np.asarray(inputs["ln1_w"], f32)
    ln1b = np.asarray(inputs["ln1_b"], f32)
    ln2w = np.asarray(inputs["ln2_w"], f32)
    ln2b = np.asarray(inputs["ln2_b"], f32)

    # fold layernorm affine params into the adjacent matmuls
    w_eff = ln1w[:, None] * attn_w               # [C, 3C]
    b_eff = attn_b + ln1b @ attn_w               # [3C]
    fcw_eff = ln2w[:, None] * fc_w               # [C, FF]
    fcb_eff = fc_b + ln2b @ fc_w                 # [FF]

    def nz(a):
        return bool(np.any(a != 0.0))

    use_bqkv = nz(b_eff)
    flags = (use_bqkv, nz(proj_b), nz(fcb_eff), nz(mlp_b), debug)

    def colsplit(v):
        return np.ascontiguousarray(v.reshape(-1, 128).T)

    bf = lambda a: np.ascontiguousarray(a).astype(NP_BF16)

    x_all = x.reshape(TT, C)
    Sx = 240.0 / max(1e-30, np.abs(x_all).max())
    xq_full = np.ascontiguousarray(
        _q8(x_all.T, Sx).reshape(NCT, 128, TT).transpose(1, 0, 2))

    Swp = 240.0 / max(1e-30, np.abs(proj_w).max())
    wp8_a = np.ascontiguousarray(
        _q8(proj_w, Swp).reshape(NCT, 128, C).transpose(1, 0, 2))

    k_idx = np.arange(128)[:, None]
    q_idx = np.arange(QCH)[None, :]
    m0 = (q_idx >= k_idx).astype(NP_BF16)
    m1 = (q_idx >= k_idx + 128).astype(NP_BF16)
    one = np.ones((128, QCH), NP_BF16)
    shared = {
        "w_fc": bf(fcw_eff), "w_mlp": bf(mlp_w),
        "b_proj": colsplit(proj_b), "b_fc": colsplit(fcb_eff),
        "b_mlp": colsplit(mlp_b),
        "wp8": wp8_a,
        "mask0": np.concatenate([m0, one, m0, one], axis=1),
        "mask1": np.concatenate([m1, one, m1, one], axis=1),
        "mask0x": np.concatenate([m0, m0], axis=1),
        "mask1x": np.concatenate([m1, m1], axis=1),
        "xq": xq_full,
    }

    in_maps = []
    for i in range(CORES):
        b, s = i // 4, i % 4
        xs = np.ascontiguousarray(x[b, s * S:(s + 1) * S, :].T)  # [C, S]
        # my heads' qkv weight columns: q, k, v blocks of 128 cols each
        cols = np.concatenate([np.arange(128) + 128 * i + blk * C
                               for blk in range(3)])
        wsl = w_eff[:, cols]                      # [C, 384]
        Sw = 240.0 / max(1e-30, np.abs(wsl).max())
        w8 = _q8(wsl, Sw)
        w8f = w8.astype(f32) / Sw
        wsum_deq = w8f.sum(axis=0)                # [384]
        bsl = b_eff[cols]
        m = dict(shared)
        m["xt32"] = xs
        m["wq8"] = np.ascontiguousarray(
            w8.reshape(NCT, 128, 384).transpose(1, 0, 2))
        m["wsum3"] = np.ascontiguousarray(wsum_deq.reshape(3, 128).T)
        m["wvsum"] = np.ascontiguousarray(wsum_deq[256:384].reshape(1, 128))
        m["b_qkv"] = np.ascontiguousarray(bsl.reshape(3, 128).T)
        m["bv_row"] = np.ascontiguousarray(bsl[256:384].reshape(1, 128))
        m["scales"] = np.array(
            [[1.0 / (Sx * Sw), 1.0 / (SY * Swp), 1.0 / (C * Sx),
              256.0 / (C * Sx * Sx), 0, 0, 0, 0]], f32)
        in_maps.append(m)
    return flags, in_maps


def run_sharded(inputs, debug=False, trace=False, trace_kwargs=None):
    flags, in_maps = _prep(inputs, debug)
    nc = _get_nc(flags)
    res = bass_utils.run_bass_kernel_spmd(
        nc, in_maps, core_ids=list(range(CORES)), trace=trace,
        **(trace_kwargs or {}))
    out = np.empty((B, T, C), np.float32)
    for i in range(CORES):
        b, s = i // 4, i % 4
        out[b, s * S:(s + 1) * S, :] = res.results[i]["out"].T
    return out, res


def kernel(**inputs):
    out, _ = run_sharded(inputs, debug=False, trace=False)
    return out


# revision 25
# speedup vs baseline: 1.0899x; 1.0317x over previous
"""Trainium2 8-core kernel for a GPT-style transformer block (v3).

Strategy:
  - Megatron-style QKV: every core holds the FULL (quantized) x and the QKV
    weight columns of its own 2 heads, so Q^T/K^T/V for all 4096 tokens are
    computed locally -- no pre-attention collectives at all.
  - LN1 is folded algebraically into the QKV matmul: qkv = rstd*(x@W) -
    mu*rstd*colsum(W). Per-token stats (sum x, sum x^2) are computed on each
    core's own 512-token span and shared with one tiny (4KB) AllGather.
  - QKV and proj matmuls run in fp8 (e4m3) DoubleRow mode: K=256 per
    instruction, 2x PE throughput. fc/mlp stay bf16 (fp8 fails tolerance).
  - Attention (2 heads/core, all tokens) processes query-chunk pairs with
    shared key tiles; softmax denominators ride along as a ones-column in V;
    normalization uses vector reciprocal (no scalar table switches).
  - y returns token-sharded via one fp8 AllToAll (0.5MB); proj + residual +
    LN2 + MLP are local to the core's 512 tokens.
"""

import sys

sys.path.insert(0, "/opt/trn_rl_repo")

import numpy as np
import ml_dtypes

import concourse.bass as bass
import concourse.mybir as mybir
import concourse.tile as tile
from concourse import bacc, bass_utils

BF16 = mybir.dt.bfloat16
F32 = mybir.dt.float32
FP8 = mybir.dt.float8e4
AF = mybir.ActivationFunctionType
ALU = mybir.AluOpType
DR = mybir.MatmulPerfMode.DoubleRow
NP_BF16 = ml_dtypes.bfloat16
NP_FP8 = ml_dtypes.float8_e4m3

B, T, C, H, HS, FF = 2, 2048, 1024, 16, 64, 4096
CORES = 8
S = 512            # tokens per core (MLP/residual shard)
TT = B * T         # 4096 total tokens
NCT = C // 128     # 8 feature tiles
NDR = NCT // 2     # 4 DoubleRow contraction pairs
NFT = FF // 128    # 32 mlp hidden tiles
QCH = 256          # query chunk
NQC = T // QCH     # 8 query chunks per batch
NKT = T // 128     # 16 key tiles per batch
NTT = TT // 128    # 32 token tiles total
EPS = 1e-5
SY = 30.0          # fixed y quantization scale (|y| < 8 by softmax convexity)


def build(flags):
    (use_bqkv, use_bproj, use_bfc, use_bmlp, debug) = flags

    nc = bacc.Bacc("TRN2", target_bir_lowering=False, debug=False,
                   num_devices=CORES)

    # ---------------- DRAM inputs ----------------
    xq = nc.dram_tensor("xq", [128, NCT, TT], FP8, kind="ExternalInput")
    xt32 = nc.dram_tensor("xt32", [C, S], F32, kind="ExternalInput")
    wq8 = nc.dram_tensor("wq8", [128, NCT, 384], FP8, kind="ExternalInput")
    wsum3 = nc.dram_tensor("wsum3", [128, 3], F32, kind="ExternalInput")
    wvsum = nc.dram_tensor("wvsum", [1, 128], F32, kind="ExternalInput")
    wp8 = nc.dram_tensor("wp8", [128, NCT, C], FP8, kind="ExternalInput")
    scales = nc.dram_tensor("scales", [1, 8], F32, kind="ExternalInput")
    b_qkv = nc.dram_tensor("b_qkv", [128, 3], F32, kind="ExternalInput")
    bv_row = nc.dram_tensor("bv_row", [1, 128], F32, kind="ExternalInput")
    w_fc = nc.dram_tensor("w_fc", [C, FF], BF16, kind="ExternalInput")
    w_mlp = nc.dram_tensor("w_mlp", [FF, C], BF16, kind="ExternalInput")
    b_proj = nc.dram_tensor("b_proj", [128, NCT], F32, kind="ExternalInput")
    b_fc = nc.dram_tensor("b_fc", [128, NFT], F32, kind="ExternalInput")
    b_mlp = nc.dram_tensor("b_mlp", [128, NCT], F32, kind="ExternalInput")
    mask0_d = nc.dram_tensor("mask0", [128, 4 * QCH], BF16, kind="ExternalInput")
    mask1_d = nc.dram_tensor("mask1", [128, 4 * QCH], BF16, kind="ExternalInput")
    mask0x_d = nc.dram_tensor("mask0x", [128, 2 * QCH], BF16, kind="ExternalInput")
    mask1x_d = nc.dram_tensor("mask1x", [128, 2 * QCH], BF16, kind="ExternalInput")
    out_d = nc.dram_tensor("out", [C, S], F32, kind="ExternalOutput")
    dbg = {}
    if debug:
        for nm, shp, dt in [("d_qt", [128, TT], BF16), ("d_kt", [128, TT], BF16),
                            ("d_v", [TT, 128], BF16), ("d_yt", [128, TT], FP8),
                            ("d_x2", [C, S], F32), ("d_ln2", [C, S], BF16)]:
            dbg[nm] = nc.dram_tensor(nm, shp, dt, kind="ExternalOutput")

    with tile.TileContext(nc) as tc:
        _build_body(nc, tc, locals(), flags)
    nc.compile()
    return nc


def _build_body(nc, tc, t_, flags):
    (use_bqkv, use_bproj, use_bfc, use_bmlp, debug) = flags
    xq, xt32 = t_["xq"], t_["xt32"]
    wq8, wsum3, wvsum, wp8 = t_["wq8"], t_["wsum3"], t_["wvsum"], t_["wp8"]
    scales, b_qkv, bv_row = t_["scales"], t_["b_qkv"], t_["bv_row"]
    w_fc, w_mlp = t_["w_fc"], t_["w_mlp"]
    b_proj, b_fc, b_mlp = t_["b_proj"], t_["b_fc"], t_["b_mlp"]
    mask0_d, mask1_d = t_["mask0_d"], t_["mask1_d"]
    mask0x_d, mask1x_d = t_["mask0x_d"], t_["mask1x_d"]
    out_d, dbg = t_["out_d"], t_["dbg"]

    from contextlib import ExitStack
    from concourse.tile import add_dep_helper

    def _delay_after(frm, to):
        f = frm.ins if hasattr(frm, "ins") else frm
        t = to.ins if hasattr(to, "ins") else to
        add_dep_helper(t, f, sync=True, reason="delay heavy DMA")
    es = ExitStack()

    consts = es.enter_context(tc.tile_pool(name="consts", bufs=1))
    dram = es.enter_context(tc.tile_pool(name="dram", bufs=1, space="DRAM"))
    # q/k/v activation tiles live from QKV production through attention
    qkvt_cm = tc.tile_pool(name="qkvt_p", bufs=1, side="right")
    qkvt_p = qkvt_cm.__enter__()  # closed manually after attention
    x2t_p = es.enter_context(tc.tile_pool(name="x2t_p", bufs=1))

    # ---- constants ----
    ones_col = consts.tile([128, 1], BF16, name="ones_col")
    nc.vector.memset(ones_col, 1.0)
    ones_row = consts.tile([1, 128], BF16, name="ones_row")
    nc.vector.memset(ones_row, 1.0)
    eps_t = consts.tile([1, 1], F32, name="eps_t")
    nc.vector.memset(eps_t, EPS)
    mask0 = consts.tile([128, 4 * QCH], BF16, name="mask0")
    nc.sync.dma_start(out=mask0, in_=mask0_d[:, :])
    mask1 = consts.tile([128, 4 * QCH], BF16, name="mask1")
    nc.sync.dma_start(out=mask1, in_=mask1_d[:, :])
    mask0x = consts.tile([128, 2 * QCH], BF16, name="mask0x")
    nc.sync.dma_start(out=mask0x, in_=mask0x_d[:, :])
    mask1x = consts.tile([128, 2 * QCH], BF16, name="mask1x")
    nc.sync.dma_start(out=mask1x, in_=mask1x_d[:, :])
    sc_sb = consts.tile([1, 8], F32, name="sc_sb")
    nc.sync.dma_start(out=sc_sb, in_=scales[:, :])
    wsum_sb = consts.tile([128, 3], F32, name="wsum_sb")
    nc.sync.dma_start(out=wsum_sb, in_=wsum3[:, :])
    wvsum_sb = consts.tile([1, 128], F32, name="wvsum_sb")
    nc.sync.dma_start(out=wvsum_sb, in_=wvsum[:, :])
    wvsum_b = consts.tile([128, 128], F32, name="wvsum_b")
    nc.gpsimd.partition_broadcast(wvsum_b, wvsum_sb)
    kproj_b = consts.tile([128, 1], F32, name="kproj_b")
    nc.gpsimd.partition_broadcast(kproj_b, sc_sb[0:1, 1:2])
    if use_bqkv:
        bqkv_sb = consts.tile([128, 3], F32, name="bqkv_sb")
        nc.sync.dma_start(out=bqkv_sb, in_=b_qkv[:, :])
        bv_sb = consts.tile([1, 128], F32, name="bv_sb")
        nc.sync.dma_start(out=bv_sb, in_=bv_row[:, :])
        bv_b = consts.tile([128, 128], F32, name="bv_b")
        nc.gpsimd.partition_broadcast(bv_b, bv_sb)
    bproj_sb = (consts.tile([128, NCT], F32, name="bproj_sb")
                if use_bproj else None)
    if use_bproj:
        nc.sync.dma_start(out=bproj_sb, in_=b_proj[:, :])
    bfc_sb = consts.tile([128, NFT], F32, name="bfc_sb") if use_bfc else None
    if use_bfc:
        nc.sync.dma_start(out=bfc_sb, in_=b_fc[:, :])
    bmlp_sb = consts.tile([128, NCT], F32, name="bmlp_sb") if use_bmlp else None
    if use_bmlp:
        nc.sync.dma_start(out=bmlp_sb, in_=b_mlp[:, :])

    # ---- collective DRAM tiles ----
    cc0_in = dram.tile([1, 128], BF16, name="cc0_in")
    cc0_out = dram.tile([CORES, 128], BF16, name="cc0_out")
    rr0b = dram.tile([1, TT], BF16, name="rr0b")
    rr1b = dram.tile([1, TT], BF16, name="rr1b")
    cc3_in = dram.tile([CORES, 128, S], FP8, name="cc3_in")
    cc3_out = dram.tile([CORES, 128, S], FP8, name="cc3_out")

    # comms warmup
    nc.sync.dma_start(out=cc0_in, in_=mask0_d[0:1, 0:128])
    nc.gpsimd.collective_compute(
        "AllGather", ALU.bypass,
        replica_groups=[list(range(CORES))],
        ins=[cc0_in[:, :].opt()],
        outs=[cc0_out[:, :].opt()])

    # persistent attention inputs
    qtb = [qkvt_p.tile([128, T], BF16, name=f"qtb_{b}") for b in range(B)]
    ktb = [qkvt_p.tile([128, T], BF16, name=f"ktb_{b}") for b in range(B)]
    vt = [qkvt_p.tile([128, 2, 128], BF16, name=f"vt_{tt}")
          for tt in range(NTT)]
    for v3 in vt:
        nc.gpsimd.memset(v3[:, :, 0:1], 1.0)
        nc.gpsimd.memset(v3[:, :, 1:64], 0.0)

    # =========================================================
    # Phase B: LN1 statistics on my span + AllGather
    # =========================================================
    qkv_cm = tc.tile_pool(name="qkv_pool", bufs=1)
    qp = qkv_cm.__enter__()
    psQ_cm = tc.tile_pool(name="psQ", bufs=1, space="PSUM")
    psQ = psQ_cm.__enter__()

    # full-token LN1 stats computed locally per chunk (no collectives on the
    # critical path: the first collective of an execution costs ~95us)
    ones8 = consts.tile([128, 2, 16], FP8, name="ones8")
    nc.vector.memset(ones8, 1.0)
    rqk_row = qp.tile([1, TT], BF16, name="rqk_row")
    nm_row = qp.tile([1, TT], BF16, name="nm_row")
    rcol_bf = qp.tile([128, NTT], BF16, name="rcol_bf")
    ncol_bf = qp.tile([128, NTT], BF16, name="ncol_bf")
    rcol = qp.tile([128, NTT], F32, name="rcol")
    ncol = qp.tile([128, NTT], F32, name="ncol")

    # =========================================================
    # Phase C: QKV in fp8 DoubleRow
    # =========================================================
    wq_sb = qp.tile([128, NCT, 384], FP8, name="wq_sb")
    nc.sync.dma_start(out=wq_sb, in_=wq8[:, :, :])

    for ch in range(NCT):
        cols = slice(ch * 512, (ch + 1) * 512)
        bi, r4 = ch // 4, ch % 4
        xc = qp.tile([128, NCT, 512], FP8, name=f"xc_{ch}", tag="xc", bufs=3)
        nc.sync.dma_start(out=xc, in_=xq[:, :, cols])
        # ---- per-chunk token stats ----
        s_ps = psQ.tile([16, 512], F32, name=f"sps_{ch}", tag="st", bufs=2)
        for d in range(NDR):
            nc.tensor.matmul(s_ps, ones8[:, :, :], xc[:, 2 * d:2 * d + 2, :],
                             start=(d == 0), stop=(d == NDR - 1),
                             perf_mode=DR)
        q_ps = psQ.tile([16, 512], F32, name=f"qps_{ch}", tag="st", bufs=2)
        for d in range(NDR):
            sq8t = qp.tile([128, 2, 512], FP8, name=f"sq_{ch}_{d}", tag="sq",
                           bufs=3)
            nc.scalar.activation(sq8t, xc[:, 2 * d:2 * d + 2, :], AF.Square,
                                 scale=1.0 / 16.0)
            nc.tensor.matmul(q_ps, ones8[:, :, :], sq8t[:, :, :],
                             start=(d == 0), stop=(d == NDR - 1),
                             perf_mode=DR)
        mu = qp.tile([1, 512], F32, name=f"mu_{ch}", tag="mu", bufs=3)
        nc.scalar.mul(mu, s_ps[0:1, :], sc_sb[0:1, 2:3])
        msq = qp.tile([1, 512], F32, name=f"msq_{ch}", tag="msq", bufs=3)
        nc.scalar.mul(msq, q_ps[0:1, :], sc_sb[0:1, 3:4])
        var = qp.tile([1, 512], F32, name=f"var_{ch}", tag="var", bufs=3)
        nc.vector.tensor_mul(var, mu, mu)
        nc.vector.tensor_sub(var, msq, var)
        lnv = qp.tile([1, 512], F32, name=f"lnv_{ch}", tag="lnv", bufs=3)
        nc.scalar.activation(lnv, var, AF.Ln, bias=eps_t, scale=1.0)
        rstd = qp.tile([1, 512], F32, name=f"rstd_{ch}", tag="rstd", bufs=3)
        nc.scalar.activation(rstd, lnv, AF.Exp, scale=-0.5)
        # exp(-lnv/2 + ln(kqk)) = rstd*kqk : dequant folded into the bias
        nc.scalar.activation(rqk_row[0:1, cols], lnv, AF.Exp,
                             bias=sc_sb[0:1, 4:5], scale=-0.5)
        nm32 = qp.tile([1, 512], F32, name=f"nm_{ch}", tag="nm32", bufs=3)
        nc.vector.tensor_mul(nm32, mu, rstd)
        nc.vector.tensor_scalar(out=nm_row[0:1, cols], in0=nm32,
                                scalar1=-1.0, scalar2=None, op0=ALU.mult)
        nc.sync.dma_start(out=rr0b[0:1, cols], in_=rqk_row[0:1, cols])
        nc.sync.dma_start(out=rr1b[0:1, cols], in_=nm_row[0:1, cols])
        rca = rr0b.rearrange("o (j p) -> (o p) j", p=128)
        nca = rr1b.rearrange("o (j p) -> (o p) j", p=128)
        csl = slice(ch * 4, (ch + 1) * 4)
        nc.sync.dma_start(out=rcol_bf[:, csl], in_=rca[:, csl])
        nc.sync.dma_start(out=ncol_bf[:, csl], in_=nca[:, csl])
        nc.vector.tensor_copy(rcol[:, csl], rcol_bf[:, csl])
        nc.vector.tensor_copy(ncol[:, csl], ncol_bf[:, csl])
        rqk_b = qp.tile([128, 512], BF16, name=f"rqkb_{ch}", tag="rqkb",
                        bufs=2)
        nc.gpsimd.partition_broadcast(rqk_b, rqk_row[0:1, cols])
        nm_b = qp.tile([128, 512], BF16, name=f"nmb_{ch}", tag="nmb", bufs=2)
        nc.gpsimd.partition_broadcast(nm_b, nm_row[0:1, cols])
        for o in range(2):  # 0=q, 1=k
            ps = psQ.tile([128, 512], F32, name=f"qk_{ch}_{o}", tag="qk",
                          bufs=3)
            for d in range(NDR):
                nc.tensor.matmul(ps,
                                 wq_sb[:, 2 * d:2 * d + 2,
                                       o * 128:(o + 1) * 128],
                                 xc[:, 2 * d:2 * d + 2, :],
                                 start=(d == 0), stop=(d == NDR - 1),
                                 perf_mode=DR)
            tmp = qp.tile([128, 512], BF16, name=f"qkt_{ch}_{o}", tag="qkt",
                          bufs=3)
            nc.vector.tensor_mul(tmp, ps, rqk_b)
            dst = (qtb if o == 0 else ktb)[bi][:, r4 * 512:(r4 + 1) * 512]
            nc.vector.scalar_tensor_tensor(
                out=dst, in0=nm_b, scalar=wsum_sb[:, o:o + 1],
                in1=tmp, op0=ALU.mult, op1=ALU.add)
            if use_bqkv:
                nc.vector.tensor_scalar(
                    out=dst, in0=dst, scalar1=bqkv_sb[:, o:o + 1],
                    scalar2=None, op0=ALU.add)
        for t4 in range(4):
            ttile = ch * 4 + t4
            tc_sl = slice(t4 * 128, (t4 + 1) * 128)
            psv = psQ.tile([128, 128], F32, name=f"vps_{ttile}", tag="vps",
                           bufs=2)
            for d in range(NDR):
                nc.tensor.matmul(psv,
                                 xc[:, 2 * d:2 * d + 2, tc_sl],
                                 wq_sb[:, 2 * d:2 * d + 2, 256:384],
                                 start=(d == 0), stop=(d == NDR - 1),
                                 perf_mode=DR)
            outer = qp.tile([128, 128], F32, name=f"outer_{ttile}",
                            tag="outer", bufs=3)
            nc.vector.tensor_scalar(
                out=outer, in0=wvsum_b, scalar1=ncol[:, ttile:ttile + 1],
                scalar2=None, op0=ALU.mult)
            if use_bqkv:
                nc.vector.tensor_add(outer, outer, bv_b)
            v3 = vt[ttile]
            for h in range(2):
                nc.vector.scalar_tensor_tensor(
                    out=v3[:, h, 64:128], in0=psv[:, h * 64:(h + 1) * 64],
                    scalar=rcol[:, ttile:ttile + 1], in1=outer[:, h * 64:(h + 1) * 64],
                    op0=ALU.mult, op1=ALU.add)
        if debug:
            for o, nm in ((0, "d_qt"), (1, "d_kt")):
                nc.sync.dma_start(
                    out=dbg[nm][:, ch * 512:(ch + 1) * 512],
                    in_=(qtb if o == 0 else ktb)[bi][:, r4 * 512:(r4 + 1) * 512])
    if debug:
        for ttile in range(NTT):
            for h in range(2):
                nc.sync.dma_start(
                    out=dbg["d_v"][ttile * 128:(ttile + 1) * 128,
                                   h * 64:(h + 1) * 64],
                    in_=vt[ttile][:, h, 64:128])
    psQ_cm.__exit__(None, None, None)
    qkv_cm.__exit__(None, None, None)

    # =========================================================
    # Phase D: attention (my 2 heads, all tokens)
    # =========================================================
    att_pool = tc.tile_pool(name="att_pool", bufs=1)
    psB_pool = tc.tile_pool(name="psB", bufs=2, space="PSUM")
    W2 = 2 * QCH
    gate_anchor = [None]
    a2a_y = None
    with att_pool as ap, psB_pool as psB:

        def flush_normalize(item):
            fb, fp, y_A, y_B = item
            j = 4 * fb + fp
            for hi, y_ps in ((0, y_A), (1, y_B)):
                # copy out of PSUM promptly so the accumulator bank frees
                ytb = ap.tile([128, W2], BF16, name=f"ytb_{fb}_{fp}_{hi}",
                              tag="ytb", bufs=4)
                nc.vector.tensor_copy(ytb[64:128, :], y_ps[64:128, :])
                rec = ap.tile([1, W2], F32, name=f"rec_{fb}_{fp}_{hi}",
                              tag="rec", bufs=4)
                nc.vector.reciprocal(rec, y_ps[0:1, :])
                rec_bf = ap.tile([1, W2], BF16, name=f"recbf_{fb}_{fp}_{hi}",
                                 tag="recbf", bufs=4)
                nc.scalar.mul(rec_bf, rec, SY)
                den = ap.tile([128, W2], BF16, name=f"den_{fb}_{fp}_{hi}",
                              tag="den", bufs=4)
                nc.gpsimd.partition_broadcast(den, rec_bf)
                yt8 = ap.tile([128, W2], FP8, name=f"yt8_{fb}_{fp}_{hi}",
                              tag="yt8", bufs=4)
                nc.vector.tensor_mul(yt8[64:128, :], ytb[64:128, :],
                                     den[64:128, :])
                nc.sync.dma_start(out=cc3_in[j, hi * 64:(hi + 1) * 64, :],
                                  in_=yt8[64:128, :])

        for b in range(B):
            for p in reversed(range(NQC // 2)):
                qc = 2 * p
                qs = qc * QCH
                nsh = 2 * (qc + 1)
                y_A = psB.tile([128, W2], F32, name=f"yA_{b}_{p}", tag="ya",
                               bufs=2)
                y_B = psB.tile([128, W2], F32, name=f"yB_{b}_{p}", tag="yb",
                               bufs=2)
                for kt in range(nsh + 2):
                    shared = kt < nsh
                    cols = slice(0, W2) if shared else slice(QCH, W2)
                    ncols = W2 if shared else QCH
                    s_AB = psB.tile([128, 2 * W2], F32, name=f"s_{b}_{p}_{kt}",
                                    tag="ps2", bufs=2)
                    nc.tensor.matmul(s_AB[:, 0:ncols],
                                     ktb[b][0:64, kt * 128:(kt + 1) * 128],
                                     qtb[b][0:64, qs + cols.start:qs + W2],
                                     start=True, stop=True)
                    nc.tensor.matmul(s_AB[:, W2:W2 + ncols],
                                     ktb[b][64:128, kt * 128:(kt + 1) * 128],
                                     qtb[b][64:128, qs + cols.start:qs + W2],
                                     start=True, stop=True)
                    e_AB = ap.tile([128, 2 * W2], BF16, name=f"e_{b}_{p}_{kt}",
                                   tag="eAB", bufs=5)
                    if shared:
                        nc.scalar.activation(e_AB, s_AB[:, :], AF.Exp,
                                             scale=1.0 / np.sqrt(HS))
                        if kt in (qc * 2, qc * 2 + 1):
                            mx = mask0x if kt == qc * 2 else mask1x
                            e4 = e_AB.rearrange("p (h c q) -> p h c q",
                                                h=2, c=2)
                            m4 = mx.rearrange("p (h q) -> p h q", h=2)
                            nc.vector.tensor_mul(e4[:, :, 0, :],
                                                 e4[:, :, 0, :], m4)
                    else:
                        e3 = e_AB.rearrange("p (h q) -> p h q", h=2)
                        s3 = s_AB.rearrange("p (h q) -> p h q", h=2)
                        nc.scalar.activation(e3[:, :, 0:QCH], s3[:, :, 0:QCH],
                                             AF.Exp, scale=1.0 / np.sqrt(HS))
                        mx = mask0x if kt == nsh else mask1x
                        nc.vector.tensor_mul(
                            e3[:, :, 0:QCH], e3[:, :, 0:QCH],
                            mx.rearrange("p (h q) -> p h q", h=2))
                    v3 = vt[b * NKT + kt]
                    nc.tensor.matmul(y_A[:, cols], v3[:, 0, :],
                                     e_AB[:, 0:ncols],
                                     start=(kt == 0), stop=(kt == nsh + 1),
                                     skip_group_check=True)
                    mmB = nc.tensor.matmul(y_B[:, cols], v3[:, 1, :],
                                           e_AB[:, W2:W2 + ncols],
                                           start=(kt == 0),
                                           stop=(kt == nsh + 1),
                                           skip_group_check=True)
                    if p == 3 and kt == nsh + 1:
                        gate_anchor.append(mmB)
                flush_normalize((b, p, y_A, y_B))
        gate_anchor[0] = gate_anchor[1]
        a2a_y = nc.gpsimd.collective_compute(
            "AllToAll", ALU.bypass,
            replica_groups=[list(range(CORES))],
            ins=[cc3_in[:, :, :].opt()],
            outs=[cc3_out[:, :, :].opt()])
    if debug:
        for j in range(CORES):
            nc.sync.dma_start(out=dbg["d_yt"][:, j * 512:(j + 1) * 512],
                              in_=cc3_in[j, :, :])
    qkvt_cm.__exit__(None, None, None)  # free qtb/ktb/vt for the MLP phase

    # =========================================================
    # Phase E: proj (fp8 DoubleRow) + residual, LN2, MLP
    # =========================================================
    mlp_pool = tc.tile_pool(name="mlp_pool", bufs=1)
    psC_cm = tc.tile_pool(name="psC", bufs=6, space="PSUM")
    psC = psC_cm.__enter__()
    with mlp_pool as mp:
        wp_sb = mp.tile([128, NCT, C], FP8, name="wp_sb")
        d = nc.sync.dma_start(out=wp_sb, in_=wp8[:, :, :])
        _delay_after(gate_anchor[0], d)
        xt32_sb = []
        for c in range(NCT):
            tl = mp.tile([128, S], F32, name=f"xt32_{c}")
            d = nc.sync.dma_start(out=tl, in_=xt32[c * 128:(c + 1) * 128, :])
            _delay_after(gate_anchor[0], d)
            xt32_sb.append(tl)
        yta8 = mp.tile([128, NCT, S], FP8, name="yta8")
        for hp in range(NCT):
            nc.sync.dma_start(out=yta8[:, hp, :], in_=cc3_out[hp])

        x2t_sb, x2bf_sb = [], []
        for co in range(NCT):
            ps = psC.tile([128, 512], F32, name=f"prps_{co}", tag="ps")
            for dd in range(NDR):
                nc.tensor.matmul(ps,
                                 wp_sb[:, 2 * dd:2 * dd + 2,
                                       co * 128:(co + 1) * 128],
                                 yta8[:, 2 * dd:2 * dd + 2, :],
                                 start=(dd == 0), stop=(dd == NDR - 1),
                                 perf_mode=DR)
            x2 = x2t_p.tile([128, S], F32, name=f"x2t_{co}")
            nc.vector.scalar_tensor_tensor(
                out=x2, in0=ps[:, :], scalar=kproj_b[:, 0:1],
                in1=xt32_sb[co], op0=ALU.mult, op1=ALU.add)
            if use_bproj:
                nc.vector.tensor_scalar(
                    out=x2, in0=x2, scalar1=bproj_sb[:, co:co + 1],
                    scalar2=None, op0=ALU.add)
            x2b = x2t_p.tile([128, S], BF16, name=f"x2bf_{co}")
            nc.vector.tensor_copy(x2b, x2)
            x2t_sb.append(x2)
            x2bf_sb.append(x2b)
            if debug:
                nc.sync.dma_start(out=dbg["d_x2"][co * 128:(co + 1) * 128, :],
                                  in_=x2)

        # ---- LN2 (always unit weight/bias: ln2 w,b folded into fc) ----
        def bcast(tag, src_bf, n):
            ps = psC.tile([128, 512], F32, name=f"{tag}_bc", tag="ps")
            nc.tensor.matmul(ps[:, :n], ones_row[:, :], src_bf[:, :n],
                             start=True, stop=True)
            return ps

        s2_ps = psC.tile([1, 512], F32, name="ln2_sps", tag="st2", bufs=2)
        q2_ps = psC.tile([1, 512], F32, name="ln2_qps", tag="st2", bufs=2)
        for c in range(NCT):
            sq = mp.tile([128, S], BF16, name=f"ln2_sq_{c}", tag="ln2_sq",
                         bufs=3)
            nc.vector.tensor_mul(sq, x2bf_sb[c], x2bf_sb[c])
            nc.tensor.matmul(s2_ps[:, :], ones_col[:, :], x2bf_sb[c][:, :],
                             start=(c == 0), stop=(c == NCT - 1))
            nc.tensor.matmul(q2_ps[:, :], ones_col[:, :], sq[:, :],
                             start=(c == 0), stop=(c == NCT - 1))
        mu_2 = mp.tile([1, S], F32, name="ln2_mu")
        nc.scalar.mul(mu_2, s2_ps[:, :], 1.0 / C)
        msq_2 = mp.tile([1, S], F32, name="ln2_msq")
        nc.scalar.mul(msq_2, q2_ps[:, :], 1.0 / C)
        mu2_2 = mp.tile([1, S], F32, name="ln2_mu2")
        nc.vector.tensor_mul(mu2_2, mu_2, mu_2)
        var_2 = mp.tile([1, S], F32, name="ln2_var")
        nc.vector.tensor_sub(var_2, msq_2, mu2_2)
        lnv_2 = mp.tile([1, S], F32, name="ln2_lnv")
        nc.scalar.activation(lnv_2, var_2, AF.Ln, bias=eps_t, scale=1.0)
        rstd_2 = mp.tile([1, S], F32, name="ln2_rstd")
        nc.scalar.activation(rstd_2, lnv_2, AF.Exp, scale=-0.5)
        rstd2_bf = mp.tile([1, S], BF16, name="ln2_rstd_bf")
        nc.vector.tensor_copy(rstd2_bf, rstd_2)
        nmurs_2 = mp.tile([1, S], F32, name="ln2_nmurs")
        nc.vector.tensor_mul(nmurs_2, mu_2, rstd_2)
        nmurs2_bf = mp.tile([1, S], BF16, name="ln2_nmurs_bf")
        nc.scalar.mul(nmurs2_bf, nmurs_2, -1.0)
        r_ps = bcast("ln2_r", rstd2_bf, S)
        sh_ps = bcast("ln2_sh", nmurs2_bf, S)
        r_b = mp.tile([128, S], BF16, name="ln2_r_b")
        nc.scalar.copy(r_b, r_ps[:, :S])
        sh_b = mp.tile([128, S], BF16, name="ln2_sh_b")
        nc.scalar.copy(sh_b, sh_ps[:, :S])
        ln2t = []
        for c in range(NCT):
            tmp = mp.tile([128, S], BF16, name=f"ln2_tmp_{c}", tag="ln2_tmp",
                          bufs=3)
            nc.vector.tensor_mul(tmp, x2bf_sb[c], r_b)
            o = mp.tile([128, S], BF16, name=f"ln2_o_{c}")
            nc.vector.tensor_add(o, tmp, sh_b)
            ln2t.append(o)
            if debug:
                nc.sync.dma_start(out=dbg["d_ln2"][c * 128:(c + 1) * 128, :],
                                  in_=o)

        # ---- fc + GELU ----
        fw_sb = {}
        for half in range(2):
            for c in range(NCT):
                tl = mp.tile([128, FF // 2], BF16, name=f"fw_{half}_{c}",
                             tag="fw", bufs=10)
                d = nc.sync.dma_start(
                    out=tl,
                    in_=w_fc[c * 128:(c + 1) * 128,
                             half * (FF // 2):(half + 1) * (FF // 2)])
                _delay_after(gate_anchor[0] if half == 0 else a2a_y, d)
                fw_sb[(half, c)] = tl
        ht = []
        for f in range(NFT):
            half, fo = f // (NFT // 2), f % (NFT // 2)
            ps = psC.tile([128, 512], F32, name=f"fcps_{f}", tag="ps")
            for c in range(NCT):
                nc.tensor.matmul(ps[:, :],
                                 fw_sb[(half, c)][:, fo * 128:(fo + 1) * 128],
                                 ln2t[c][:, :],
                                 start=(c == 0), stop=(c == NCT - 1))
            h = mp.tile([128, S], BF16, name=f"ht_{f}")
            if use_bfc:
                nc.scalar.activation(h, ps[:, :], AF.Gelu,
                                     bias=bfc_sb[:, f:f + 1], scale=1.0)
            else:
                nc.scalar.activation(h, ps[:, :], AF.Gelu, scale=1.0)
            ht.append(h)

        # ---- mlp proj + residual -> out ----
        psC_cm.__exit__(None, None, None)
        psM_cm = tc.tile_pool(name="psM", bufs=8, space="PSUM")
        psM = psM_cm.__enter__()
        accs = [psM.tile([128, 512], F32, name=f"mlps_{co}", tag="psm",
                         bufs=8) for co in range(NCT)]
        for f in range(NFT):
            tl = mp.tile([128, C], BF16, name=f"mw_{f}", tag="mw", bufs=3)
            d = nc.sync.dma_start(out=tl, in_=w_mlp[f * 128:(f + 1) * 128, :])
            _delay_after(a2a_y, d)
            for co in range(NCT):
                nc.tensor.matmul(accs[co][:, :],
                                 tl[:, co * 128:(co + 1) * 128],
                                 ht[f][:, :],
                                 start=(f == 0), stop=(f == NFT - 1))
        for co in range(NCT):
            o = mp.tile([128, S], F32, name=f"out_{co}", tag="outt", bufs=3)
            if use_bmlp:
                nc.vector.scalar_tensor_tensor(
                    out=o, in0=accs[co][:, :], scalar=bmlp_sb[:, co:co + 1],
                    in1=x2t_sb[co], op0=ALU.add, op1=ALU.add)
            else:
                nc.vector.tensor_add(o, accs[co][:, :], x2t_sb[co])
            nc.sync.dma_start(out=out_d[co * 128:(co + 1) * 128, :], in_=o)
        psM_cm.__exit__(None, None, None)

    es.close()


# =============================================================
# Host side
# =============================================================
_CACHE = {}


def _get_nc(flags):
    if flags not in _CACHE:
        _CACHE[flags] = build(flags)
    return _CACHE[flags]


def _q8(a, scale):
    return np.clip(np.asarray(a, np.float32) * scale,
                   -240.0, 240.0).astype(NP_FP8)


def _prep(inputs, debug=False):
    f32 = np.float32
    x = np.asarray(inputs["x"], f32)
    attn_w = np.asarray(inputs["attn_w"], f32)
    attn_b = np.asarray(inputs["attn_b"], f32)
    proj_w = np.asarray(inputs["proj_w"], f32)
    proj_b = np.asarray(inputs["proj_b"], f32)
    fc_w = np.asarray(inputs["fc_w"], f32)
    fc_b = np.asarray(inputs["fc_b"], f32)
    mlp_w = np.asarray(inputs["mlp_proj_w"], f32)
    mlp_b = np.asarray(inputs["mlp_proj_b"], f32)
    ln1w = np.asarray(inputs["ln1_w"], f32)
    ln1b = np.asarray(inputs["ln1_b"], f32)
    ln2w = np.asarray(inputs["ln2_w"], f32)
    ln2b = np.asarray(inputs["ln2_b"], f32)

    # fold layernorm affine params into the adjacent matmuls
    w_eff = ln1w[:, None] * attn_w               # [C, 3C]
    b_eff = attn_b + ln1b @ attn_w               # [3C]
    fcw_eff = ln2w[:, None] * fc_w               # [C, FF]
    fcb_eff = fc_b + ln2b @ fc_w                 # [FF]

    def nz(a):
        return bool(np.any(a != 0.0))

    use_bqkv = nz(b_eff)
    flags = (use_bqkv, nz(proj_b), nz(fcb_eff), nz(mlp_b), debug)

    def colsplit(v):
        return np.ascontiguousarray(v.reshape(-1, 128).T)

    bf = lambda a: np.ascontiguousarray(a).astype(NP_BF16)

    x_all = x.reshape(TT, C)
    Sx = 240.0 / max(1e-30, np.abs(x_all).max())
    xq_full = np.ascontiguousarray(
        _q8(x_all.T, Sx).reshape(NCT, 128, TT).transpose(1, 0, 2))

    Swp = 240.0 / max(1e-30, np.abs(proj_w).max())
    wp8_a = np.ascontiguousarray(
        _q8(proj_w, Swp).reshape(NCT, 128, C).transpose(1, 0, 2))

    k_idx = np.arange(128)[:, None]
    q_idx = np.arange(QCH)[None, :]
    m0 = (q_idx >= k_idx).astype(NP_BF16)
    m1 = (q_idx >= k_idx + 128).astype(NP_BF16)
    one = np.ones((128, QCH), NP_BF16)
    shared = {
        "w_fc": bf(fcw_eff), "w_mlp": bf(mlp_w),
        "b_proj": colsplit(proj_b), "b_fc": colsplit(fcb_eff),
        "b_mlp": colsplit(mlp_b),
        "wp8": wp8_a,
        "mask0": np.concatenate([m0, one, m0, one], axis=1),
        "mask1": np.concatenate([m1, one, m1, one], axis=1),
        "mask0x": np.concatenate([m0, m0], axis=1),
        "mask1x": np.concatenate([m1, m1], axis=1),
        "xq": xq_full,
    }

    in_maps = []
    for i in range(CORES):
        b, s = i // 4, i % 4
        xs = np.ascontiguousarray(x[b, s * S:(s + 1) * S, :].T)  # [C, S]
        # my heads' qkv weight columns: q, k, v blocks of 128 cols each
        cols = np.concatenate([np.arange(128) + 128 * i + blk * C
                               for blk in range(3)])
        wsl = w_eff[:, cols]                      # [C, 384]
        Sw = 240.0 / max(1e-30, np.abs(wsl).max())
        w8 = _q8(wsl, Sw)
        w8f = w8.astype(f32) / Sw
        wsum_deq = w8f.sum(axis=0)                # [384]
        bsl = b_eff[cols]
        m = dict(shared)
        m["xt32"] = xs
        m["wq8"] = np.ascontiguousarray(
            w8.reshape(NCT, 128, 384).transpose(1, 0, 2))
        m["wsum3"] = np.ascontiguousarray(wsum_deq.reshape(3, 128).T)
        m["wvsum"] = np.ascontiguousarray(wsum_deq[256:384].reshape(1, 128))
        m["b_qkv"] = np.ascontiguousarray(bsl.reshape(3, 128).T)
        m["bv_row"] = np.ascontiguousarray(bsl[256:384].reshape(1, 128))
        m["scales"] = np.array(
            [[1.0 / (Sx * Sw), 1.0 / (SY * Swp), 1.0 / (C * Sx),
              256.0 / (C * Sx * Sx), -np.log(Sx * Sw), 0, 0, 0]], f32)
        in_maps.append(m)
    return flags, in_maps


def run_sharded(inputs, debug=False, trace=False, trace_kwargs=None):
    flags, in_maps = _prep(inputs, debug)
    nc = _get_nc(flags)
    res = bass_utils.run_bass_kernel_spmd(
        nc, in_maps, core_ids=list(range(CORES)), trace=trace,
        **(trace_kwargs or {}))
    out = np.empty((B, T, C), np.float32)
    for i in range(CORES):
        b, s = i // 4, i % 4
        out[b, s * S:(s + 1) * S, :] = res.results[i]["out"].T
    return out, res


def kernel(**inputs):
    out, _ = run_sharded(inputs, debug=False, trace=False)
    return out
